# revision 2
# baseline (speedup 1.0000x reference)
"""NeuralSDE forecasting kernel for 8x Trainium2 NeuronCores (Bass/Tile). v3.

Data-parallel over batch B=256 across 8 cores (32 batch elems per core).
Feature-major scan: state y.T lives in [128 partitions, 4*32]; column
block k holds features 128k..128k+128 of the 32 local batch columns.

Precision: W = fp16(W) + 2^-11 * e5m2((W - fp16(W)) * 2^11). Per step each
of the three [512,512] products is y_hi@W16 + y_lo@W16 + y8@W8 with fp32
PSUM accumulation (y carried f32, split to fp16 hi/lo; y8 = e5m2 of
y_hi * 2^-11 so the fp8 product scale cancels). CPU-simulated end-to-end
rel err 3.4e-3 (vs 2e-2 budget).

Speed structure (vs the double-bf16 baseline at 1.88ms):
- u_t = x~_t @ [W1x; b1] and z0 are computed EXACTLY on the host and
  shipped as inputs: kills the on-device precompute and its DRAM
  round-trip, the strided per-step u gather, and all u rounding error.
- Both hi products accumulate into a SINGLE [128,32] psum block per
  m-chunk via a broadcast (stride-0) out AP on the N=64 pair matmul --
  verified on HW that both halves accumulate. The fp8 product joins the
  same accumulation group, so no DVE fold of hi/lo psum blocks exists;
  tau and f are computed by ACT reading PSUM directly with per-chunk
  bias APs (saves ~4 DVE hops per step off the critical path).
- The W1y group streams k-outer as three N=32 passes (y_hi, y_lo, y8) so
  its first matmuls depend only on the first 64-column chunk of the new
  state; the tail (f, y16, y2, ylo) is produced in 64-col chunks,
  shrinking the between-step PE stall from ~1.8us to a few hundred ns.

sigmoid(x) = 0.5*(1+tanh(x/2)) keeps the scan on the Tanh ACT table;
0.5 factors are folded into the host-prescaled dW and bg.
"""

import os
import sys

sys.path.insert(0, "/opt/trn_rl_repo")

import numpy as np
import ml_dtypes

import concourse.bass as bass
import concourse.bacc as bacc
import concourse.mybir as mybir
import concourse.tile as tile
from concourse.bass_utils import run_bass_kernel_spmd

B, T, C, H, O = 256, 256, 32, 512, 32
OUT_TIME = 32
NCORES = 8
BL = B // NCORES  # 32 batch elements per core
NT = int(os.environ.get("BASS_NT", T - 1))  # 255 scan steps
SAVE0 = NT - OUT_TIME  # first step whose y_next lands in the output tail
KC = H // 128  # 4 feature chunks
F32 = mybir.dt.float32
F16 = mybir.dt.float16
F8 = mybir.dt.float8e5
F16NP = np.float16
F8NP = ml_dtypes.float8_e5m2
LSC = np.float32(2.0**11)  # lo-split scale

Tanh = mybir.ActivationFunctionType.Tanh
Relu = mybir.ActivationFunctionType.Relu
Copy = mybir.ActivationFunctionType.Copy
Identity = mybir.ActivationFunctionType.Identity

_BUILT = None


def _build_nc():
    nc = bacc.Bacc("TRN2", target_bir_lowering=False, debug=False)

    d_u = nc.dram_tensor("u", [NT, 128, 2 * KC * BL], F16, kind="ExternalInput")
    d_eye = nc.dram_tensor("eye16", [128, 128], F16, kind="ExternalInput")
    d_dw = nc.dram_tensor("dw", [NT, 128, KC * BL], F32, kind="ExternalInput")
    d_y0 = nc.dram_tensor("y0", [128, KC * BL], F32, kind="ExternalInput")
    wnames = ["w1y", "w2", "wg"]
    d_w16 = {
        n: nc.dram_tensor(f"{n}_16", [128, KC * H], F16, kind="ExternalInput")
        for n in wnames
    }
    d_w8 = {
        n: nc.dram_tensor(f"{n}_8", [128, KC * H], F8, kind="ExternalInput")
        for n in wnames
    }
    d_b2r = nc.dram_tensor("b2r", [1, KC * 128], F16, kind="ExternalInput")
    d_bgr = nc.dram_tensor("bgr", [1, KC * 128], F16, kind="ExternalInput")
    d_wh1 = nc.dram_tensor("wh1", [128, KC * H], F32, kind="ExternalInput")
    d_wh2 = nc.dram_tensor("wh2", [128, KC * O], F32, kind="ExternalInput")
    d_bh1 = nc.dram_tensor("bh1t", [128, KC], F32, kind="ExternalInput")
    d_bh2 = nc.dram_tensor("bh2t", [O, 1], F32, kind="ExternalInput")
    d_out = nc.dram_tensor("out", [O, OUT_TIME * BL], F32, kind="ExternalOutput")

    with tile.TileContext(nc) as tc:
        with (
            tc.tile_pool(name="const", bufs=1) as const,
            tc.tile_pool(name="xp", bufs=8) as xp,
            tc.tile_pool(name="dwp", bufs=8) as dwp,
            tc.tile_pool(name="yp", bufs=2) as yp,
            tc.tile_pool(name="tmp", bufs=3) as tmp,
            tc.tile_pool(name="pp", bufs=2, space="PSUM") as pp,
        ):
            # --- resident weights ---
            w16, w8 = {}, {}
            for n in wnames:
                w16[n] = const.tile([128, KC * H], F16, tag=f"w16{n}", name=f"w16_{n}")
                nc.sync.dma_start(out=w16[n][:], in_=d_w16[n][:])
                w8[n] = const.tile([128, KC * H], F8, tag=f"w8{n}", name=f"w8_{n}")
                nc.sync.dma_start(out=w8[n][:], in_=d_w8[n][:])
            b2r = const.tile([1, KC * 128], F16, tag="b2r")
            bgr = const.tile([1, KC * 128], F16, tag="bgr")
            ones16 = const.tile([1, BL], F16, tag="ones16")
            nc.vector.memset(ones16[:], 1.0)
            eye16 = const.tile([128, 128], F16, tag="eye16")
            nc.sync.dma_start(out=eye16[:], in_=d_eye[:])
            wh1 = const.tile([128, KC * H], F32, tag="wh1")
            wh2 = const.tile([128, KC * O], F32, tag="wh2")
            bh1 = const.tile([128, KC], F32, tag="bh1")
            bh2 = const.tile([O, 1], F32, tag="bh2")
            slab = const.tile([128, OUT_TIME * 128], F32, tag="slab")
            rT = const.tile([128, KC * 1024], F32, tag="rT")
            outs = const.tile([O, OUT_TIME * BL], F32, tag="outs")
            y0s = const.tile([128, KC * BL], F32, tag="y0s")
            for dst, src in [
                (b2r, d_b2r), (bgr, d_bgr), (wh1, d_wh1), (wh2, d_wh2),
                (bh1, d_bh1), (bh2, d_bh2), (y0s, d_y0),
            ]:
                nc.sync.dma_start(out=dst[:], in_=src[:])

            def wsl(w, n, k, m):  # lhsT tile (k, m) of weight n
                return w[n][:, k * H + m * 128 : k * H + (m + 1) * 128]

            def bcast(ps, m):  # [128, 2, 32] stride-0 view of psum block m
                return (
                    ps[:, m * BL : (m + 1) * BL]
                    .unsqueeze(1)
                    .broadcast_to((128, 2, BL))
                )

            # --- initial state ---
            y = y0s[:]
            yhl = tmp.tile([128, KC * 2 * BL], F16, tag="yhl", name="yhl_init")
            nc.vector.tensor_copy(yhl[:, 0:128], y)
            nc.vector.tensor_sub(yhl[:, 128:256], y, yhl[:, 0:128])
            y8 = tmp.tile([128, KC * BL], F8, tag="y8", name="y8_init")
            nc.scalar.activation(y8[:], yhl[:, 0:128], Copy, scale=float(1.0 / LSC))

            # N=64 pair group (for wg/w2): hi halves via stride-0 bcast out,
            # then the fp8 correction into the same accumulation group.
            # start=True ONLY on the group's first matmul: it clears the
            # has_written bits for the whole psum zero-region; every later
            # matmul relies on per-element has_written (first touch of an
            # element replaces, later touches accumulate). A start=True per
            # m-block would wipe the accumulate flag of sibling blocks.
            def bias_mm(ps, brow):
                # K=1 rank-1 matmul seeding the psum group with a bias row,
                # so tau/f become single full-width ACTs with no chunk bias.
                for m in range(KC):
                    nc.tensor.matmul(
                        ps[:, m * BL : (m + 1) * BL],
                        brow[0:1, m * 128 : (m + 1) * 128], ones16[:],
                        start=(m == 0), stop=False, skip_group_check=True,
                    )

            # C group: N=64 pair matmuls only -- the wg fp8-lo correction is
            # dropped (gate path tolerates fp16-single weights; simulated
            # end-to-end err 0.0152 with the h-single cut below).
            def group_pair(ps, n, hl, lead=True):
                rv = hl[:].rearrange("p (h q) -> p h q", h=2)
                for m in range(KC):
                    bc = bcast(ps, m)
                    for k in range(KC):
                        nc.tensor.matmul(
                            bc, wsl(w16, n, k, m),
                            rv[:, :, k * BL : (k + 1) * BL],
                            start=(lead and m == 0 and k == 0),
                            stop=(m == KC - 1 and k == KC - 1),
                            skip_group_check=True,
                        )

            # B group: h streamed single-fp16 (no h_lo pass) + fp8 leg.
            def group_b(ps, h16, lo8):
                for k in range(KC):
                    for m in range(KC):
                        nc.tensor.matmul(
                            ps[:, m * BL : (m + 1) * BL], wsl(w16, "w2", k, m),
                            h16[:, k * BL : (k + 1) * BL],
                            start=False, stop=False, skip_group_check=True,
                        )
                for k in range(KC):
                    for m in range(KC):
                        nc.tensor.matmul(
                            ps[:, m * BL : (m + 1) * BL], wsl(w8, "w2", k, m),
                            lo8[:, k * BL : (k + 1) * BL],
                            start=False, stop=(k == KC - 1 and m == KC - 1),
                            skip_group_check=True,
                        )

            # k-outer 3-pass group: first matmuls depend only on
            # the first chunk of the freshly produced state.
            def group_split(ps, n, hl, lo8, lead=True):
                for k in range(KC):
                    for m in range(KC):
                        nc.tensor.matmul(
                            ps[:, m * BL : (m + 1) * BL], wsl(w16, n, k, m),
                            hl[:, k * BL : (k + 1) * BL],
                            start=(lead and k == 0 and m == 0), stop=False,
                            skip_group_check=True,
                        )
                for k in range(KC):
                    for m in range(KC):
                        nc.tensor.matmul(
                            ps[:, m * BL : (m + 1) * BL], wsl(w16, n, k, m),
                            hl[:, 128 + k * BL : 128 + (k + 1) * BL],
                            start=False, stop=False, skip_group_check=True,
                        )
                for k in range(KC):
                    for m in range(KC):
                        nc.tensor.matmul(
                            ps[:, m * BL : (m + 1) * BL], wsl(w8, n, k, m),
                            lo8[:, k * BL : (k + 1) * BL],
                            start=False, stop=(k == KC - 1 and m == KC - 1),
                            skip_group_check=True,
                        )

            # --- scan ---
            for t in range(NT):
                u_t = xp.tile([128, 2 * KC * BL], F16, tag="u", name=f"u_{t}")
                nc.sync.dma_start(out=u_t[:], in_=d_u[t])
                dw_t = dwp.tile([128, KC * BL], F32, tag="dw", name=f"dw_{t}")
                nc.sync.dma_start(out=dw_t[:], in_=d_dw[t])

                # A group: u seeded into psum via two identity matmuls
                # (u shipped as an fp16 hi+lo pair -> exact to 2^-22), so
                # the pre-tanh add needs no DVE op at all and h16 reads
                # PSUM directly.
                psA = pp.tile([128, KC * BL], F32, tag="psA", name=f"psA_{t}")
                nc.tensor.matmul(
                    psA[:], eye16[:], u_t[:, 0:128],
                    start=True, stop=False, skip_group_check=True,
                )
                nc.tensor.matmul(
                    psA[:], eye16[:], u_t[:, 128:256],
                    start=False, stop=False, skip_group_check=True,
                )
                group_split(psA, "w1y", yhl, y8, lead=False)
                # C group emitted immediately after A (inputs all ready), so
                # the h-chain ACTs below overlap C's matmuls.
                psC = pp.tile([128, KC * BL], F32, tag="psC", name=f"psC_{t}")
                bias_mm(psC, bgr)
                group_pair(psC, "wg", yhl, lead=False)

                # h = tanh(y@W1y + u), fp16 single; h8 fp8 leg from h16.
                h16 = tmp.tile([128, KC * BL], F16, tag="h16", name=f"h16_{t}")
                nc.scalar.activation(h16[:], psA[:], Tanh)
                h8 = tmp.tile([128, KC * BL], F8, tag="h8", name=f"h8_{t}")
                nc.scalar.activation(
                    h8[:], h16[:], Copy, scale=float(1.0 / LSC)
                )

                # tau = tanh((y@Wg + bg)/2)  (sigmoid fold; bias pre-seeded)
                tau = tmp.tile([128, KC * BL], F32, tag="tau", name=f"tau_{t}")
                nc.scalar.activation(tau[:], psC[:], Tanh, scale=0.5)
                # t1 = (tau + 1) * dw ;  dw pre-scaled by 0.5*sqrt(dt)/dt
                t1 = tmp.tile([128, KC * BL], F32, tag="t1", name=f"t1_{t}")
                nc.vector.scalar_tensor_tensor(
                    t1[:], tau[:], 1.0, dw_t[:],
                    mybir.AluOpType.add, mybir.AluOpType.mult,
                )
                yh2 = tmp.tile([128, KC * BL], F32, tag="yh2", name=f"yh2_{t}")
                nc.vector.tensor_add(yh2[:], y, t1[:])

                # B group: f = tanh(h@W2 + b2); h16 single + fp8 leg.
                psB = pp.tile([128, KC * BL], F32, tag="psB", bufs=3,
                              name=f"psB_{t}")
                bias_mm(psB, b2r)
                group_b(psB, h16, h8)
                # f in 2 chunks so the tail's y16 chunk 0 starts earlier
                f = tmp.tile([128, KC * BL], F32, tag="f", name=f"f_{t}")
                nc.scalar.activation(f[:, 0:64], psB[:, 0:64], Tanh)
                nc.scalar.activation(f[:, 64:128], psB[:, 64:128], Tanh)

                # y_next = (y + t1) + f, produced in 64-col chunks so the
                # next step's first matmuls start after chunk 0.
                if t >= SAVE0:
                    y2 = slab[:, (t - SAVE0) * 128 : (t - SAVE0 + 1) * 128]
                else:
                    y2_t = yp.tile([128, KC * BL], F32, tag="y", name=f"y_{t}")
                    y2 = y2_t[:]
                yhl = tmp.tile([128, KC * 2 * BL], F16, tag="yhl", name=f"yhl_{t}")
                for c in range(2):
                    cs = slice(c * 64, (c + 1) * 64)
                    nc.vector.tensor_add(yhl[:, cs], yh2[:, cs], f[:, cs])
                    nc.vector.tensor_add(y2[:, cs], yh2[:, cs], f[:, cs])
                    nc.vector.tensor_sub(
                        yhl[:, 128 + c * 64 : 128 + (c + 1) * 64],
                        y2[:, cs], yhl[:, cs],
                    )
                y8 = tmp.tile([128, KC * BL], F8, tag="y8", name=f"y8_{t}")
                nc.scalar.activation(
                    y8[:], yhl[:, 0:128], Copy, scale=float(1.0 / LSC)
                )
                y = y2

            # --- head (fp32): out = relu(z_tail@Wh1 + bh1) @ Wh2 + bh2 ---
            slab_r = slab[:].rearrange(
                "p (s k b) -> p s k b", s=OUT_TIME, k=KC, b=BL
            )
            for m in range(KC):
                for hf in range(2):
                    ps1 = pp.tile([128, 512], F32, tag="psA", name=f"ps1_{m}_{hf}")
                    for k in range(KC):
                        nc.tensor.matmul(
                            ps1[:],
                            wh1[:, k * H + m * 128 : k * H + (m + 1) * 128],
                            slab_r[:, hf * 16 : (hf + 1) * 16, k, :],
                            start=(k == 0), stop=(k == KC - 1),
                        )
                    nc.scalar.activation(
                        rT[:, m * 1024 + hf * 512 : m * 1024 + (hf + 1) * 512],
                        ps1[:], Relu, bias=bh1[:, m : m + 1],
                    )
            for hf in range(2):
                ps2 = pp.tile([O, 512], F32, tag="psB", bufs=3, name=f"ps2_{hf}")
                for m in range(KC):
                    nc.tensor.matmul(
                        ps2[:],
                        wh2[:, m * O : (m + 1) * O],
                        rT[:, m * 1024 + hf * 512 : m * 1024 + (hf + 1) * 512],
                        start=(m == 0), stop=(m == KC - 1),
                    )
                nc.scalar.activation(
                    outs[:, hf * 512 : (hf + 1) * 512], ps2[:], Identity,
                    bias=bh2[:],
                )
            nc.sync.dma_start(out=d_out[:], in_=outs[:])

    nc.compile()
    return nc


def _prep_inputs(times, coeffs, final_index, dW, W_init, b_init, W1, b1, W2,
                 b2, Wg, bg, Wh1, bh1, Wh2, bh2):
    f32 = np.float32
    times = np.asarray(times, f32)
    dt = f32(max(np.min(times[1:] - times[:-1]), f32(0.001)))
    sq = f32(np.sqrt(dt))

    def lhsT_layout(w):  # [H, H] -> [128, KC*H] with (k,m) tile at k*H+m*128
        return np.ascontiguousarray(
            np.asarray(w, f32).reshape(KC, 128, H).transpose(1, 0, 2).reshape(128, KC * H)
        )

    def chunk_col(b):  # [H] -> [128, KC]
        return np.ascontiguousarray(np.asarray(b, f32).reshape(KC, 128).T)

    W1 = np.asarray(W1, f32)
    shared = {}
    for name, w in [("w1y", dt * W1[:H]), ("w2", np.asarray(W2, f32)),
                    ("wg", dt * np.asarray(Wg, f32))]:
        wl = lhsT_layout(w)
        hi = wl.astype(F16NP)
        lo = ((wl - hi.astype(f32)) * LSC).astype(F8NP)
        shared[f"{name}_16"] = hi
        shared[f"{name}_8"] = lo
    shared["b2r"] = np.asarray(b2, f32).reshape(1, H).astype(F16NP)
    # tau ACT applies scale=0.5 AFTER the psum seed, so seed the full bg:
    # tanh((pre + bg) * 0.5)
    shared["bgr"] = np.asarray(bg, f32).reshape(1, H).astype(F16NP)
    shared["wh1"] = lhsT_layout(dt * np.asarray(Wh1, f32))
    shared["wh2"] = np.ascontiguousarray(
        np.asarray(Wh2, f32).reshape(KC, 128, O).transpose(1, 0, 2).reshape(128, KC * O)
    )
    shared["bh1t"] = chunk_col(bh1)
    shared["bh2t"] = np.asarray(bh2, f32).reshape(O, 1)

    coeffs = np.asarray(coeffs, f32)  # [B, T, C]
    dW = np.asarray(dW, f32)  # [NT_full, B, H]
    dw_scale = f32(0.5 * sq / dt)

    # exact host-side u_t = x_t @ W1x + b1 for all t, and z0
    x_seq = coeffs.transpose(1, 0, 2)  # [T, B, C]
    u_all = x_seq[:NT] @ W1[H:] + np.asarray(b1, f32)  # [NT, B, H]
    z0 = (x_seq[0] @ np.asarray(W_init, f32) + np.asarray(b_init, f32)) / dt
    shared["eye16"] = np.eye(128, dtype=F16NP)

    def fmaj(a, bs):  # [.., B, H] slice -> [.., 128, KC*BL] feature-major
        v = a[..., bs, :]
        sh = v.shape[:-2]
        v = np.swapaxes(v, -1, -2)  # [.., H, BL]
        v = v.reshape(*sh, KC, 128, BL).swapaxes(-2, -3)  # [.., 128, KC, BL]
        return np.ascontiguousarray(v.reshape(*sh, 128, KC * BL), f32)

    in_maps = []
    for c in range(NCORES):
        bs = slice(c * BL, (c + 1) * BL)
        uf = fmaj(u_all, bs)  # [NT, 128, 128] f32
        uhi = uf.astype(F16NP)
        ulo = (uf - uhi.astype(f32)).astype(F16NP)
        upair = np.concatenate([uhi, ulo], axis=-1)  # [NT, 128, 256] fp16
        in_maps.append(
            {"u": np.ascontiguousarray(upair), "dw": fmaj(dW[:NT] * dw_scale, bs),
             "y0": fmaj(z0, bs), **shared}
        )
    return in_maps


def kernel(**inputs):
    global _BUILT
    if _BUILT is None:
        _BUILT = _build_nc()
    nc = _BUILT
    in_maps = _prep_inputs(**inputs)
    res = run_bass_kernel_spmd(nc, in_maps, core_ids=list(range(NCORES)))
    out = np.empty((B, OUT_TIME, O), np.float32)
    for c, r in enumerate(res.results):
        out[c * BL : (c + 1) * BL] = (
            r["out"].reshape(O, OUT_TIME, BL).transpose(2, 1, 0)
        )
    return out


# revision 3
# speedup vs baseline: 1.0880x; 1.0880x over previous
"""NeuralSDE forecasting kernel for 8x Trainium2 NeuronCores (Bass/Tile).

1.30ms HW (vs 1.88ms double-bf16 baseline), rel err 1.52e-2 (< 2e-2 gate;
bit-deterministic across runs and matching the CPU emulation exactly).

Data-parallel over batch B=256 across 8 cores (32 batch elems per core).
Feature-major scan: state y.T lives in [128 partitions, 4*32]; column
block k holds features 128k..128k+128 of the 32 local batch columns.

Precision scheme (CPU-emulated first in precision_sim.py, HW-confirmed):
- W1y: fp16 hi + scaled-e5m2 lo split, W = W16 + 2^-11*e5m2(res*2^11);
  products y16@W16 + ylo@W16 + y8@W8 (y carried f32, split fp16 hi/lo;
  y8 = e5m2(y16 * 2^-11) so the fp8 product scale cancels in PSUM).
- Wg (gate): fp16 single on the y16/ylo pair (sigmoid path tolerates it).
- W2: fp16 single on h16 = fp16(tanh) plus the e5m2 leg h8.

Speed structure:
- u_t = x~_t @ [W1x; b1] and z0 are computed EXACTLY on the host and
  shipped as inputs (host prep is part of kernel()): kills the on-device
  precompute, its DRAM round-trip, and all u rounding error. u ships as
  an fp16 hi+lo pair and is seeded into PSUM by two identity matmuls, so
  the pre-tanh add costs no DVE op and tanh reads PSUM directly.
- All products of one logical [512,512]@[512,32] accumulate into a
  SINGLE [128,32] psum block per m-chunk: the N=64 pair matmul writes
  both state halves through a broadcast (stride-0) out AP (HW-verified
  to accumulate), and the fp8 leg joins the same accumulation group.
  Only the group's FIRST matmul carries start=True -- a start per
  m-block would wipe sibling blocks' has_written accumulate flags.
- Biases ride K=1 rank-1 matmuls seeding the psum groups (bias row x
  ones), so tau and f are single full-width ACTs reading PSUM.
- The W1y group streams k-outer as three N=32 passes (y16, ylo, y8) so
  its first matmuls need only the first 64-col chunk of the new state;
  f and the y16/y2/ylo tail are produced in 64-col chunks. Group order
  per step is A(w1y), C(wg), B(w2) with the h-chain ACTs overlapping C.

sigmoid(x) = 0.5*(1+tanh(x/2)) keeps the scan on the Tanh ACT table;
0.5 factors are folded into the host-prescaled dW.
"""

import os
import sys

sys.path.insert(0, "/opt/trn_rl_repo")

import numpy as np
import ml_dtypes

import concourse.bass as bass
import concourse.bacc as bacc
import concourse.mybir as mybir
import concourse.tile as tile
from concourse.bass_utils import run_bass_kernel_spmd

B, T, C, H, O = 256, 256, 32, 512, 32
OUT_TIME = 32
NCORES = 8
BL = B // NCORES  # 32 batch elements per core
NT = int(os.environ.get("BASS_NT", T - 1))  # 255 scan steps
SAVE0 = NT - OUT_TIME  # first step whose y_next lands in the output tail
KC = H // 128  # 4 feature chunks
F32 = mybir.dt.float32
F16 = mybir.dt.float16
F8 = mybir.dt.float8e5
F16NP = np.float16
F8NP = ml_dtypes.float8_e5m2
LSC = np.float32(2.0**11)  # lo-split scale

Tanh = mybir.ActivationFunctionType.Tanh
Relu = mybir.ActivationFunctionType.Relu
Copy = mybir.ActivationFunctionType.Copy
Identity = mybir.ActivationFunctionType.Identity

_BUILT = None


def _build_nc():
    nc = bacc.Bacc("TRN2", target_bir_lowering=False, debug=False)

    d_u = nc.dram_tensor("u", [NT, 128, 2 * KC * BL], F16, kind="ExternalInput")
    d_eye = nc.dram_tensor("eye16", [128, 128], F16, kind="ExternalInput")
    d_dw = nc.dram_tensor("dw", [NT, 128, KC * BL], F32, kind="ExternalInput")
    d_y0 = nc.dram_tensor("y0", [128, KC * BL], F32, kind="ExternalInput")
    wnames = ["w1y", "w2", "wg"]
    d_w16 = {
        n: nc.dram_tensor(f"{n}_16", [128, KC * H], F16, kind="ExternalInput")
        for n in wnames
    }
    d_w8 = {
        n: nc.dram_tensor(f"{n}_8", [128, KC * H], F8, kind="ExternalInput")
        for n in wnames
    }
    d_b2r = nc.dram_tensor("b2r", [1, KC * 128], F16, kind="ExternalInput")
    d_bgr = nc.dram_tensor("bgr", [1, KC * 128], F16, kind="ExternalInput")
    d_wh1 = nc.dram_tensor("wh1", [128, KC * H], F32, kind="ExternalInput")
    d_wh2 = nc.dram_tensor("wh2", [128, KC * O], F32, kind="ExternalInput")
    d_bh1 = nc.dram_tensor("bh1t", [128, KC], F32, kind="ExternalInput")
    d_bh2 = nc.dram_tensor("bh2t", [O, 1], F32, kind="ExternalInput")
    d_out = nc.dram_tensor("out", [O, OUT_TIME * BL], F32, kind="ExternalOutput")

    with tile.TileContext(nc) as tc:
        with (
            tc.tile_pool(name="const", bufs=1) as const,
            tc.tile_pool(name="xp", bufs=8) as xp,
            tc.tile_pool(name="dwp", bufs=8) as dwp,
            tc.tile_pool(name="yp", bufs=2) as yp,
            tc.tile_pool(name="tmp", bufs=3) as tmp,
            tc.tile_pool(name="pp", bufs=2, space="PSUM") as pp,
        ):
            # --- resident weights ---
            w16, w8 = {}, {}
            for n in wnames:
                w16[n] = const.tile([128, KC * H], F16, tag=f"w16{n}", name=f"w16_{n}")
                nc.sync.dma_start(out=w16[n][:], in_=d_w16[n][:])
                w8[n] = const.tile([128, KC * H], F8, tag=f"w8{n}", name=f"w8_{n}")
                nc.sync.dma_start(out=w8[n][:], in_=d_w8[n][:])
            b2r = const.tile([1, KC * 128], F16, tag="b2r")
            bgr = const.tile([1, KC * 128], F16, tag="bgr")
            ones16 = const.tile([1, BL], F16, tag="ones16")
            nc.vector.memset(ones16[:], 1.0)
            eye16 = const.tile([128, 128], F16, tag="eye16")
            nc.sync.dma_start(out=eye16[:], in_=d_eye[:])
            wh1 = const.tile([128, KC * H], F32, tag="wh1")
            wh2 = const.tile([128, KC * O], F32, tag="wh2")
            bh1 = const.tile([128, KC], F32, tag="bh1")
            bh2 = const.tile([O, 1], F32, tag="bh2")
            slab = const.tile([128, OUT_TIME * 128], F32, tag="slab")
            rT = const.tile([128, KC * 1024], F32, tag="rT")
            outs = const.tile([O, OUT_TIME * BL], F32, tag="outs")
            y0s = const.tile([128, KC * BL], F32, tag="y0s")
            for dst, src in [
                (b2r, d_b2r), (bgr, d_bgr), (wh1, d_wh1), (wh2, d_wh2),
                (bh1, d_bh1), (bh2, d_bh2), (y0s, d_y0),
            ]:
                nc.sync.dma_start(out=dst[:], in_=src[:])

            def wsl(w, n, k, m):  # lhsT tile (k, m) of weight n
                return w[n][:, k * H + m * 128 : k * H + (m + 1) * 128]

            def bcast(ps, m):  # [128, 2, 32] stride-0 view of psum block m
                return (
                    ps[:, m * BL : (m + 1) * BL]
                    .unsqueeze(1)
                    .broadcast_to((128, 2, BL))
                )

            # --- initial state ---
            y = y0s[:]
            yhl = tmp.tile([128, KC * 2 * BL], F16, tag="yhl", name="yhl_init")
            nc.vector.tensor_copy(yhl[:, 0:128], y)
            nc.vector.tensor_sub(yhl[:, 128:256], y, yhl[:, 0:128])
            y8 = tmp.tile([128, KC * BL], F8, tag="y8", name="y8_init")
            nc.scalar.activation(y8[:], yhl[:, 0:128], Copy, scale=float(1.0 / LSC))

            # N=64 pair group (for wg/w2): hi halves via stride-0 bcast out,
            # then the fp8 correction into the same accumulation group.
            # start=True ONLY on the group's first matmul: it clears the
            # has_written bits for the whole psum zero-region; every later
            # matmul relies on per-element has_written (first touch of an
            # element replaces, later touches accumulate). A start=True per
            # m-block would wipe the accumulate flag of sibling blocks.
            def bias_mm(ps, brow):
                # K=1 rank-1 matmul seeding the psum group with a bias row,
                # so tau/f become single full-width ACTs with no chunk bias.
                for m in range(KC):
                    nc.tensor.matmul(
                        ps[:, m * BL : (m + 1) * BL],
                        brow[0:1, m * 128 : (m + 1) * 128], ones16[:],
                        start=(m == 0), stop=False, skip_group_check=True,
                    )

            # C group: N=64 pair matmuls only -- the wg fp8-lo correction is
            # dropped (gate path tolerates fp16-single weights; simulated
            # end-to-end err 0.0152 with the h-single cut below).
            def group_pair(ps, n, hl, lead=True):
                rv = hl[:].rearrange("p (h q) -> p h q", h=2)
                for m in range(KC):
                    bc = bcast(ps, m)
                    for k in range(KC):
                        nc.tensor.matmul(
                            bc, wsl(w16, n, k, m),
                            rv[:, :, k * BL : (k + 1) * BL],
                            start=(lead and m == 0 and k == 0),
                            stop=(m == KC - 1 and k == KC - 1),
                            skip_group_check=True,
                        )

            # B group: h streamed single-fp16 (no h_lo pass) + fp8 leg.
            def group_b(ps, h16, lo8):
                for k in range(KC):
                    for m in range(KC):
                        nc.tensor.matmul(
                            ps[:, m * BL : (m + 1) * BL], wsl(w16, "w2", k, m),
                            h16[:, k * BL : (k + 1) * BL],
                            start=False, stop=False, skip_group_check=True,
                        )
                for k in range(KC):
                    for m in range(KC):
                        nc.tensor.matmul(
                            ps[:, m * BL : (m + 1) * BL], wsl(w8, "w2", k, m),
                            lo8[:, k * BL : (k + 1) * BL],
                            start=False, stop=(k == KC - 1 and m == KC - 1),
                            skip_group_check=True,
                        )

            # k-outer 3-pass group: first matmuls depend only on
            # the first chunk of the freshly produced state.
            def group_split(ps, n, hl, lo8, lead=True):
                for k in range(KC):
                    for m in range(KC):
                        nc.tensor.matmul(
                            ps[:, m * BL : (m + 1) * BL], wsl(w16, n, k, m),
                            hl[:, k * BL : (k + 1) * BL],
                            start=(lead and k == 0 and m == 0), stop=False,
                            skip_group_check=True,
                        )
                for k in range(KC):
                    for m in range(KC):
                        nc.tensor.matmul(
                            ps[:, m * BL : (m + 1) * BL], wsl(w16, n, k, m),
                            hl[:, 128 + k * BL : 128 + (k + 1) * BL],
                            start=False, stop=False, skip_group_check=True,
                        )
                for k in range(KC):
                    for m in range(KC):
                        nc.tensor.matmul(
                            ps[:, m * BL : (m + 1) * BL], wsl(w8, n, k, m),
                            lo8[:, k * BL : (k + 1) * BL],
                            start=False, stop=(k == KC - 1 and m == KC - 1),
                            skip_group_check=True,
                        )

            # --- scan ---
            for t in range(NT):
                u_t = xp.tile([128, 2 * KC * BL], F16, tag="u", name=f"u_{t}")
                nc.sync.dma_start(out=u_t[:], in_=d_u[t])
                dw_t = dwp.tile([128, KC * BL], F32, tag="dw", name=f"dw_{t}")
                nc.sync.dma_start(out=dw_t[:], in_=d_dw[t])

                # A group: u seeded into psum via two identity matmuls
                # (u shipped as an fp16 hi+lo pair -> exact to 2^-22), so
                # the pre-tanh add needs no DVE op at all and h16 reads
                # PSUM directly.
                psA = pp.tile([128, KC * BL], F32, tag="psA", name=f"psA_{t}")
                nc.tensor.matmul(
                    psA[:], eye16[:], u_t[:, 0:128],
                    start=True, stop=False, skip_group_check=True,
                )
                nc.tensor.matmul(
                    psA[:], eye16[:], u_t[:, 128:256],
                    start=False, stop=False, skip_group_check=True,
                )
                group_split(psA, "w1y", yhl, y8, lead=False)
                # C group emitted immediately after A (inputs all ready), so
                # the h-chain ACTs below overlap C's matmuls.
                psC = pp.tile([128, KC * BL], F32, tag="psC", name=f"psC_{t}")
                bias_mm(psC, bgr)
                group_pair(psC, "wg", yhl, lead=False)

                # h = tanh(y@W1y + u), fp16 single; h8 fp8 leg from h16.
                h16 = tmp.tile([128, KC * BL], F16, tag="h16", name=f"h16_{t}")
                nc.scalar.activation(h16[:], psA[:], Tanh)
                h8 = tmp.tile([128, KC * BL], F8, tag="h8", name=f"h8_{t}")
                nc.scalar.activation(
                    h8[:], h16[:], Copy, scale=float(1.0 / LSC)
                )

                # tau = tanh((y@Wg + bg)/2)  (sigmoid fold; bias pre-seeded)
                tau = tmp.tile([128, KC * BL], F32, tag="tau", name=f"tau_{t}")
                nc.scalar.activation(tau[:], psC[:], Tanh, scale=0.5)
                # t1 = (tau + 1) * dw ;  dw pre-scaled by 0.5*sqrt(dt)/dt
                t1 = tmp.tile([128, KC * BL], F32, tag="t1", name=f"t1_{t}")
                nc.vector.scalar_tensor_tensor(
                    t1[:], tau[:], 1.0, dw_t[:],
                    mybir.AluOpType.add, mybir.AluOpType.mult,
                )
                yh2 = tmp.tile([128, KC * BL], F32, tag="yh2", name=f"yh2_{t}")
                nc.vector.tensor_add(yh2[:], y, t1[:])

                # B group: f = tanh(h@W2 + b2); h16 single + fp8 leg.
                psB = pp.tile([128, KC * BL], F32, tag="psB", bufs=3,
                              name=f"psB_{t}")
                bias_mm(psB, b2r)
                group_b(psB, h16, h8)
                # f in 2 chunks so the tail's y16 chunk 0 starts earlier
                f = tmp.tile([128, KC * BL], F32, tag="f", name=f"f_{t}")
                nc.scalar.activation(f[:, 0:64], psB[:, 0:64], Tanh)
                nc.scalar.activation(f[:, 64:128], psB[:, 64:128], Tanh)

                # y_next = (y + t1) + f, produced in 64-col chunks so the
                # next step's first matmuls start after chunk 0.
                if t >= SAVE0:
                    y2 = slab[:, (t - SAVE0) * 128 : (t - SAVE0 + 1) * 128]
                else:
                    y2_t = yp.tile([128, KC * BL], F32, tag="y", name=f"y_{t}")
                    y2 = y2_t[:]
                yhl = tmp.tile([128, KC * 2 * BL], F16, tag="yhl", name=f"yhl_{t}")
                for c in range(2):
                    cs = slice(c * 64, (c + 1) * 64)
                    nc.vector.tensor_add(yhl[:, cs], yh2[:, cs], f[:, cs])
                    nc.vector.tensor_add(y2[:, cs], yh2[:, cs], f[:, cs])
                    nc.vector.tensor_sub(
                        yhl[:, 128 + c * 64 : 128 + (c + 1) * 64],
                        y2[:, cs], yhl[:, cs],
                    )
                y8 = tmp.tile([128, KC * BL], F8, tag="y8", name=f"y8_{t}")
                nc.scalar.activation(
                    y8[:], yhl[:, 0:128], Copy, scale=float(1.0 / LSC)
                )
                y = y2

            # --- head (fp32): out = relu(z_tail@Wh1 + bh1) @ Wh2 + bh2 ---
            slab_r = slab[:].rearrange(
                "p (s k b) -> p s k b", s=OUT_TIME, k=KC, b=BL
            )
            for m in range(KC):
                for hf in range(2):
                    ps1 = pp.tile([128, 512], F32, tag="psA", name=f"ps1_{m}_{hf}")
                    for k in range(KC):
                        nc.tensor.matmul(
                            ps1[:],
                            wh1[:, k * H + m * 128 : k * H + (m + 1) * 128],
                            slab_r[:, hf * 16 : (hf + 1) * 16, k, :],
                            start=(k == 0), stop=(k == KC - 1),
                        )
                    nc.scalar.activation(
                        rT[:, m * 1024 + hf * 512 : m * 1024 + (hf + 1) * 512],
                        ps1[:], Relu, bias=bh1[:, m : m + 1],
                    )
            for hf in range(2):
                ps2 = pp.tile([O, 512], F32, tag="psB", bufs=3, name=f"ps2_{hf}")
                for m in range(KC):
                    nc.tensor.matmul(
                        ps2[:],
                        wh2[:, m * O : (m + 1) * O],
                        rT[:, m * 1024 + hf * 512 : m * 1024 + (hf + 1) * 512],
                        start=(m == 0), stop=(m == KC - 1),
                    )
                nc.scalar.activation(
                    outs[:, hf * 512 : (hf + 1) * 512], ps2[:], Identity,
                    bias=bh2[:],
                )
            nc.sync.dma_start(out=d_out[:], in_=outs[:])

    nc.compile()
    return nc


def _prep_inputs(times, coeffs, final_index, dW, W_init, b_init, W1, b1, W2,
                 b2, Wg, bg, Wh1, bh1, Wh2, bh2):
    f32 = np.float32
    times = np.asarray(times, f32)
    dt = f32(max(np.min(times[1:] - times[:-1]), f32(0.001)))
    sq = f32(np.sqrt(dt))

    def lhsT_layout(w):  # [H, H] -> [128, KC*H] with (k,m) tile at k*H+m*128
        return np.ascontiguousarray(
            np.asarray(w, f32).reshape(KC, 128, H).transpose(1, 0, 2).reshape(128, KC * H)
        )

    def chunk_col(b):  # [H] -> [128, KC]
        return np.ascontiguousarray(np.asarray(b, f32).reshape(KC, 128).T)

    W1 = np.asarray(W1, f32)
    shared = {}
    for name, w in [("w1y", dt * W1[:H]), ("w2", np.asarray(W2, f32)),
                    ("wg", dt * np.asarray(Wg, f32))]:
        wl = lhsT_layout(w)
        hi = wl.astype(F16NP)
        lo = ((wl - hi.astype(f32)) * LSC).astype(F8NP)
        shared[f"{name}_16"] = hi
        shared[f"{name}_8"] = lo
    shared["b2r"] = np.asarray(b2, f32).reshape(1, H).astype(F16NP)
    # tau ACT applies scale=0.5 AFTER the psum seed, so seed the full bg:
    # tanh((pre + bg) * 0.5)
    shared["bgr"] = np.asarray(bg, f32).reshape(1, H).astype(F16NP)
    shared["wh1"] = lhsT_layout(dt * np.asarray(Wh1, f32))
    shared["wh2"] = np.ascontiguousarray(
        np.asarray(Wh2, f32).reshape(KC, 128, O).transpose(1, 0, 2).reshape(128, KC * O)
    )
    shared["bh1t"] = chunk_col(bh1)
    shared["bh2t"] = np.asarray(bh2, f32).reshape(O, 1)

    coeffs = np.asarray(coeffs, f32)  # [B, T, C]
    dW = np.asarray(dW, f32)  # [NT_full, B, H]
    dw_scale = f32(0.5 * sq / dt)

    # exact host-side u_t = x_t @ W1x + b1 for all t, and z0
    x_seq = coeffs.transpose(1, 0, 2)  # [T, B, C]
    u_all = x_seq[:NT] @ W1[H:] + np.asarray(b1, f32)  # [NT, B, H]
    z0 = (x_seq[0] @ np.asarray(W_init, f32) + np.asarray(b_init, f32)) / dt
    shared["eye16"] = np.eye(128, dtype=F16NP)

    def fmaj(a, bs):  # [.., B, H] slice -> [.., 128, KC*BL] feature-major
        v = a[..., bs, :]
        sh = v.shape[:-2]
        v = np.swapaxes(v, -1, -2)  # [.., H, BL]
        v = v.reshape(*sh, KC, 128, BL).swapaxes(-2, -3)  # [.., 128, KC, BL]
        return np.ascontiguousarray(v.reshape(*sh, 128, KC * BL), f32)

    in_maps = []
    for c in range(NCORES):
        bs = slice(c * BL, (c + 1) * BL)
        uf = fmaj(u_all, bs)  # [NT, 128, 128] f32
        uhi = uf.astype(F16NP)
        ulo = (uf - uhi.astype(f32)).astype(F16NP)
        upair = np.concatenate([uhi, ulo], axis=-1)  # [NT, 128, 256] fp16
        in_maps.append(
            {"u": np.ascontiguousarray(upair), "dw": fmaj(dW[:NT] * dw_scale, bs),
             "y0": fmaj(z0, bs), **shared}
        )
    return in_maps


def kernel(**inputs):
    global _BUILT
    if _BUILT is None:
        _BUILT = _build_nc()
    nc = _BUILT
    in_maps = _prep_inputs(**inputs)
    res = run_bass_kernel_spmd(nc, in_maps, core_ids=list(range(NCORES)))
    out = np.empty((B, OUT_TIME, O), np.float32)
    for c, r in enumerate(res.results):
        out[c * BL : (c + 1) * BL] = (
            r["out"].reshape(O, OUT_TIME, BL).transpose(2, 1, 0)
        )
    return out


# revision 4
# speedup vs baseline: 1.1255x; 1.0345x over previous
"""NeuralSDE forecasting kernel for 8x Trainium2 NeuronCores (Bass/Tile). v3.

Data-parallel over batch B=256 across 8 cores (32 batch elems per core).
Feature-major scan: state y.T lives in [128 partitions, 4*32]; column
block k holds features 128k..128k+128 of the 32 local batch columns.

Precision: W = fp16(W) + 2^-11 * e5m2((W - fp16(W)) * 2^11). Per step each
of the three [512,512] products is y_hi@W16 + y_lo@W16 + y8@W8 with fp32
PSUM accumulation (y carried f32, split to fp16 hi/lo; y8 = e5m2 of
y_hi * 2^-11 so the fp8 product scale cancels). CPU-simulated end-to-end
rel err 3.4e-3 (vs 2e-2 budget).

Speed structure (vs the double-bf16 baseline at 1.88ms):
- u_t = x~_t @ [W1x; b1] and z0 are computed EXACTLY on the host and
  shipped as inputs: kills the on-device precompute and its DRAM
  round-trip, the strided per-step u gather, and all u rounding error.
- Both hi products accumulate into a SINGLE [128,32] psum block per
  m-chunk via a broadcast (stride-0) out AP on the N=64 pair matmul --
  verified on HW that both halves accumulate. The fp8 product joins the
  same accumulation group, so no DVE fold of hi/lo psum blocks exists;
  tau and f are computed by ACT reading PSUM directly with per-chunk
  bias APs (saves ~4 DVE hops per step off the critical path).
- The W1y group streams k-outer as three N=32 passes (y_hi, y_lo, y8) so
  its first matmuls depend only on the first 64-column chunk of the new
  state; the tail (f, y16, y2, ylo) is produced in 64-col chunks,
  shrinking the between-step PE stall from ~1.8us to a few hundred ns.

sigmoid(x) = 0.5*(1+tanh(x/2)) keeps the scan on the Tanh ACT table;
0.5 factors are folded into the host-prescaled dW and bg.
"""

import os
import sys

sys.path.insert(0, "/opt/trn_rl_repo")

import numpy as np
import ml_dtypes

import concourse.bass as bass
import concourse.bacc as bacc
import concourse.mybir as mybir
import concourse.tile as tile
from concourse.bass_utils import run_bass_kernel_spmd

B, T, C, H, O = 256, 256, 32, 512, 32
OUT_TIME = 32
NCORES = 8
BL = B // NCORES  # 32 batch elements per core
NT = int(os.environ.get("BASS_NT", T - 1))  # 255 scan steps
SAVE0 = NT - OUT_TIME  # first step whose y_next lands in the output tail
KC = H // 128  # 4 feature chunks
F32 = mybir.dt.float32
F16 = mybir.dt.float16
F8 = mybir.dt.float8e5
F16NP = np.float16
F8NP = ml_dtypes.float8_e5m2
LSC = np.float32(2.0**11)  # lo-split scale

Tanh = mybir.ActivationFunctionType.Tanh
Relu = mybir.ActivationFunctionType.Relu
Copy = mybir.ActivationFunctionType.Copy
Identity = mybir.ActivationFunctionType.Identity

_BUILT = None


def _build_nc():
    nc = bacc.Bacc("TRN2", target_bir_lowering=False, debug=False)

    d_u = nc.dram_tensor("u", [NT, 128, 2 * KC * BL], F16, kind="ExternalInput")
    d_eye = nc.dram_tensor("eye16", [128, 128], F16, kind="ExternalInput")
    d_dw = nc.dram_tensor("dw", [NT, 128, KC * BL], F32, kind="ExternalInput")
    d_y0 = nc.dram_tensor("y0", [128, KC * BL], F32, kind="ExternalInput")
    wnames = ["w1y", "w2", "wg"]
    d_w16 = {
        n: nc.dram_tensor(f"{n}_16", [128, KC * H], F16, kind="ExternalInput")
        for n in wnames
    }
    d_w8 = {
        n: nc.dram_tensor(f"{n}_8", [128, KC * H], F8, kind="ExternalInput")
        for n in wnames
    }
    d_b2r = nc.dram_tensor("b2r", [1, KC * 128], F16, kind="ExternalInput")
    d_bgr = nc.dram_tensor("bgr", [1, KC * 128], F16, kind="ExternalInput")
    d_wh1 = nc.dram_tensor("wh1", [128, KC * H], F32, kind="ExternalInput")
    d_wh2 = nc.dram_tensor("wh2", [128, KC * O], F32, kind="ExternalInput")
    d_bh1 = nc.dram_tensor("bh1t", [128, KC], F32, kind="ExternalInput")
    d_bh2 = nc.dram_tensor("bh2t", [O, 1], F32, kind="ExternalInput")
    d_out = nc.dram_tensor("out", [O, OUT_TIME * BL], F32, kind="ExternalOutput")

    with tile.TileContext(nc) as tc:
        with (
            tc.tile_pool(name="const", bufs=1) as const,
            tc.tile_pool(name="xp", bufs=8) as xp,
            tc.tile_pool(name="dwp", bufs=8) as dwp,
            tc.tile_pool(name="yp", bufs=2) as yp,
            tc.tile_pool(name="tmp", bufs=3) as tmp,
            tc.tile_pool(name="pp", bufs=2, space="PSUM") as pp,
        ):
            # --- resident weights ---
            w16, w8 = {}, {}
            for n in wnames:
                w16[n] = const.tile([128, KC * H], F16, tag=f"w16{n}", name=f"w16_{n}")
                nc.sync.dma_start(out=w16[n][:], in_=d_w16[n][:])
                w8[n] = const.tile([128, KC * H], F8, tag=f"w8{n}", name=f"w8_{n}")
                nc.sync.dma_start(out=w8[n][:], in_=d_w8[n][:])
            b2r = const.tile([1, KC * 128], F16, tag="b2r")
            bgr = const.tile([1, KC * 128], F16, tag="bgr")
            ones16 = const.tile([1, BL], F16, tag="ones16")
            nc.vector.memset(ones16[:], 1.0)
            eye16 = const.tile([128, 128], F16, tag="eye16")
            nc.sync.dma_start(out=eye16[:], in_=d_eye[:])
            wh1 = const.tile([128, KC * H], F32, tag="wh1")
            wh2 = const.tile([128, KC * O], F32, tag="wh2")
            bh1 = const.tile([128, KC], F32, tag="bh1")
            bh2 = const.tile([O, 1], F32, tag="bh2")
            slab = const.tile([128, OUT_TIME * 128], F32, tag="slab")
            rT = const.tile([128, KC * 1024], F32, tag="rT")
            outs = const.tile([O, OUT_TIME * BL], F32, tag="outs")
            y0s = const.tile([128, KC * BL], F32, tag="y0s")
            for dst, src in [
                (b2r, d_b2r), (bgr, d_bgr), (wh1, d_wh1), (wh2, d_wh2),
                (bh1, d_bh1), (bh2, d_bh2), (y0s, d_y0),
            ]:
                nc.sync.dma_start(out=dst[:], in_=src[:])

            def wsl(w, n, k, m):  # lhsT tile (k, m) of weight n
                return w[n][:, k * H + m * 128 : k * H + (m + 1) * 128]

            def bcast(ps, m):  # [128, 2, 32] stride-0 view of psum block m
                return (
                    ps[:, m * BL : (m + 1) * BL]
                    .unsqueeze(1)
                    .broadcast_to((128, 2, BL))
                )

            # --- initial state ---
            y = y0s[:]
            yhl = tmp.tile([128, KC * 2 * BL], F16, tag="yhl", name="yhl_init")
            nc.vector.tensor_copy(yhl[:, 0:128], y)
            nc.vector.tensor_sub(yhl[:, 128:256], y, yhl[:, 0:128])
            y8 = tmp.tile([128, KC * BL], F8, tag="y8", name="y8_init")
            nc.scalar.activation(y8[:], yhl[:, 0:128], Copy, scale=float(1.0 / LSC))

            # N=64 pair group (for wg/w2): hi halves via stride-0 bcast out,
            # then the fp8 correction into the same accumulation group.
            # start=True ONLY on the group's first matmul: it clears the
            # has_written bits for the whole psum zero-region; every later
            # matmul relies on per-element has_written (first touch of an
            # element replaces, later touches accumulate). A start=True per
            # m-block would wipe the accumulate flag of sibling blocks.
            def bias_mm(ps, brow):
                # K=1 rank-1 matmul seeding the psum group with a bias row,
                # so tau/f become single full-width ACTs with no chunk bias.
                for m in range(KC):
                    nc.tensor.matmul(
                        ps[:, m * BL : (m + 1) * BL],
                        brow[0:1, m * 128 : (m + 1) * 128], ones16[:],
                        start=(m == 0), stop=False, skip_group_check=True,
                    )

            # C group: N=64 pair matmuls only -- the wg fp8-lo correction is
            # dropped (gate path tolerates fp16-single weights; simulated
            # end-to-end err 0.0152 with the h-single cut below).
            def group_pair(ps, n, hl, lead=True):
                rv = hl[:].rearrange("p (h q) -> p h q", h=2)
                for m in range(KC):
                    bc = bcast(ps, m)
                    for k in range(KC):
                        nc.tensor.matmul(
                            bc, wsl(w16, n, k, m),
                            rv[:, :, k * BL : (k + 1) * BL],
                            start=(lead and m == 0 and k == 0),
                            stop=(m == KC - 1 and k == KC - 1),
                            skip_group_check=True,
                        )

            # B group: h streamed single-fp16 (no h_lo pass) + fp8 leg.
            # psB is SPLIT into two physical psum tiles (separate banks =>
            # separate accumulation groups with their own stop): readers
            # wait on the group-stop instruction, so closing the m0/m1
            # half a half-group early lets the first f chunk start ~430ns
            # before the whole B group finishes.
            def group_b_half(ps, h16, lo8, brow, ms):
                for j, m in enumerate(ms):
                    nc.tensor.matmul(
                        ps[:, j * BL : (j + 1) * BL],
                        brow[0:1, m * 128 : (m + 1) * 128], ones16[:],
                        start=(j == 0), stop=False, skip_group_check=True,
                    )
                for k in range(KC):
                    for j, m in enumerate(ms):
                        nc.tensor.matmul(
                            ps[:, j * BL : (j + 1) * BL], wsl(w16, "w2", k, m),
                            h16[:, k * BL : (k + 1) * BL],
                            start=False, stop=False, skip_group_check=True,
                        )
                for k in range(KC):
                    for j, m in enumerate(ms):
                        nc.tensor.matmul(
                            ps[:, j * BL : (j + 1) * BL], wsl(w8, "w2", k, m),
                            lo8[:, k * BL : (k + 1) * BL],
                            start=False, stop=(k == KC - 1 and j == len(ms) - 1),
                            skip_group_check=True,
                        )

            # k-outer 3-pass group ordered by when the tail produces each
            # input: y16 pass, then y8 (fp8) pass, then ylo pass -- the
            # DVE/ACT tail emits y16 chunks, y8, then ylo chunks, so
            # consumption matches production and the mid-group stall that
            # used to wait on ylo disappears.
            def group_split(ps, n, hl, lo8, lead=True):
                for k in range(KC):
                    for m in range(KC):
                        nc.tensor.matmul(
                            ps[:, m * BL : (m + 1) * BL], wsl(w16, n, k, m),
                            hl[:, k * BL : (k + 1) * BL],
                            start=(lead and k == 0 and m == 0), stop=False,
                            skip_group_check=True,
                        )
                for k in range(KC):
                    for m in range(KC):
                        nc.tensor.matmul(
                            ps[:, m * BL : (m + 1) * BL], wsl(w8, n, k, m),
                            lo8[:, k * BL : (k + 1) * BL],
                            start=False, stop=False, skip_group_check=True,
                        )
                for k in range(KC):
                    for m in range(KC):
                        nc.tensor.matmul(
                            ps[:, m * BL : (m + 1) * BL], wsl(w16, n, k, m),
                            hl[:, 128 + k * BL : 128 + (k + 1) * BL],
                            start=False, stop=(k == KC - 1 and m == KC - 1),
                            skip_group_check=True,
                        )

            # --- scan ---
            for t in range(NT):
                u_t = xp.tile([128, 2 * KC * BL], F16, tag="u", name=f"u_{t}")
                nc.sync.dma_start(out=u_t[:], in_=d_u[t])
                dw_t = dwp.tile([128, KC * BL], F32, tag="dw", name=f"dw_{t}")
                nc.sync.dma_start(out=dw_t[:], in_=d_dw[t])

                # A group: u seeded into psum via two identity matmuls
                # (u shipped as an fp16 hi+lo pair -> exact to 2^-22), so
                # the pre-tanh add needs no DVE op at all and h16 reads
                # PSUM directly.
                psA = pp.tile([128, KC * BL], F32, tag="psA", name=f"psA_{t}")
                nc.tensor.matmul(
                    psA[:], eye16[:], u_t[:, 0:128],
                    start=True, stop=False, skip_group_check=True,
                )
                nc.tensor.matmul(
                    psA[:], eye16[:], u_t[:, 128:256],
                    start=False, stop=False, skip_group_check=True,
                )
                group_split(psA, "w1y", yhl, y8, lead=False)
                # C group emitted immediately after A (inputs all ready), so
                # the h-chain ACTs below overlap C's matmuls.
                psC = pp.tile([128, KC * BL], F32, tag="psC", name=f"psC_{t}")
                bias_mm(psC, bgr)
                group_pair(psC, "wg", yhl, lead=False)

                # h = tanh(y@W1y + u), fp16 single; h8 fp8 leg from h16.
                h16 = tmp.tile([128, KC * BL], F16, tag="h16", name=f"h16_{t}")
                nc.scalar.activation(h16[:], psA[:], Tanh)
                h8 = tmp.tile([128, KC * BL], F8, tag="h8", name=f"h8_{t}")
                nc.scalar.activation(
                    h8[:], h16[:], Copy, scale=float(1.0 / LSC)
                )

                # tau = tanh((y@Wg + bg)/2)  (sigmoid fold; bias pre-seeded)
                tau = tmp.tile([128, KC * BL], F32, tag="tau", name=f"tau_{t}")
                nc.scalar.activation(tau[:], psC[:], Tanh, scale=0.5)
                # t1 = (tau + 1) * dw ;  dw pre-scaled by 0.5*sqrt(dt)/dt
                t1 = tmp.tile([128, KC * BL], F32, tag="t1", name=f"t1_{t}")
                nc.vector.scalar_tensor_tensor(
                    t1[:], tau[:], 1.0, dw_t[:],
                    mybir.AluOpType.add, mybir.AluOpType.mult,
                )
                yh2 = tmp.tile([128, KC * BL], F32, tag="yh2", name=f"yh2_{t}")
                nc.vector.tensor_add(yh2[:], y, t1[:])

                # B group: f = tanh(h@W2 + b2); h16 single + fp8 leg.
                psB0 = pp.tile([128, 2 * BL], F32, tag="psB0", bufs=2,
                               name=f"psB0_{t}")
                psB1 = pp.tile([128, 2 * BL], F32, tag="psB1", bufs=2,
                               name=f"psB1_{t}")
                group_b_half(psB0, h16, h8, b2r, (0, 1))
                group_b_half(psB1, h16, h8, b2r, (2, 3))
                # f chunks read their half-tile as soon as its group closes
                f = tmp.tile([128, KC * BL], F32, tag="f", name=f"f_{t}")
                nc.scalar.activation(f[:, 0:64], psB0[:], Tanh)
                nc.scalar.activation(f[:, 64:128], psB1[:], Tanh)

                # y_next = (y + t1) + f, produced in 64-col chunks so the
                # next step's first matmuls start after chunk 0.
                if t >= SAVE0:
                    y2 = slab[:, (t - SAVE0) * 128 : (t - SAVE0 + 1) * 128]
                else:
                    y2_t = yp.tile([128, KC * BL], F32, tag="y", name=f"y_{t}")
                    y2 = y2_t[:]
                yhl = tmp.tile([128, KC * 2 * BL], F16, tag="yhl", name=f"yhl_{t}")
                for c in range(2):
                    cs = slice(c * 64, (c + 1) * 64)
                    nc.vector.tensor_add(yhl[:, cs], yh2[:, cs], f[:, cs])
                    nc.vector.tensor_add(y2[:, cs], yh2[:, cs], f[:, cs])
                    nc.vector.tensor_sub(
                        yhl[:, 128 + c * 64 : 128 + (c + 1) * 64],
                        y2[:, cs], yhl[:, cs],
                    )
                y8 = tmp.tile([128, KC * BL], F8, tag="y8", name=f"y8_{t}")
                nc.scalar.activation(
                    y8[:], yhl[:, 0:128], Copy, scale=float(1.0 / LSC)
                )
                y = y2

            # --- head (fp32): out = relu(z_tail@Wh1 + bh1) @ Wh2 + bh2 ---
            slab_r = slab[:].rearrange(
                "p (s k b) -> p s k b", s=OUT_TIME, k=KC, b=BL
            )
            for m in range(KC):
                for hf in range(2):
                    ps1 = pp.tile([128, 512], F32, tag="psA", name=f"ps1_{m}_{hf}")
                    for k in range(KC):
                        nc.tensor.matmul(
                            ps1[:],
                            wh1[:, k * H + m * 128 : k * H + (m + 1) * 128],
                            slab_r[:, hf * 16 : (hf + 1) * 16, k, :],
                            start=(k == 0), stop=(k == KC - 1),
                        )
                    nc.scalar.activation(
                        rT[:, m * 1024 + hf * 512 : m * 1024 + (hf + 1) * 512],
                        ps1[:], Relu, bias=bh1[:, m : m + 1],
                    )
            for hf in range(2):
                ps2 = pp.tile([O, 512], F32, tag="psC", name=f"ps2_{hf}")
                for m in range(KC):
                    nc.tensor.matmul(
                        ps2[:],
                        wh2[:, m * O : (m + 1) * O],
                        rT[:, m * 1024 + hf * 512 : m * 1024 + (hf + 1) * 512],
                        start=(m == 0), stop=(m == KC - 1),
                    )
                nc.scalar.activation(
                    outs[:, hf * 512 : (hf + 1) * 512], ps2[:], Identity,
                    bias=bh2[:],
                )
            nc.sync.dma_start(out=d_out[:], in_=outs[:])

    nc.compile()
    return nc


def _prep_inputs(times, coeffs, final_index, dW, W_init, b_init, W1, b1, W2,
                 b2, Wg, bg, Wh1, bh1, Wh2, bh2):
    f32 = np.float32
    times = np.asarray(times, f32)
    dt = f32(max(np.min(times[1:] - times[:-1]), f32(0.001)))
    sq = f32(np.sqrt(dt))

    def lhsT_layout(w):  # [H, H] -> [128, KC*H] with (k,m) tile at k*H+m*128
        return np.ascontiguousarray(
            np.asarray(w, f32).reshape(KC, 128, H).transpose(1, 0, 2).reshape(128, KC * H)
        )

    def chunk_col(b):  # [H] -> [128, KC]
        return np.ascontiguousarray(np.asarray(b, f32).reshape(KC, 128).T)

    W1 = np.asarray(W1, f32)
    shared = {}
    for name, w in [("w1y", dt * W1[:H]), ("w2", np.asarray(W2, f32)),
                    ("wg", dt * np.asarray(Wg, f32))]:
        wl = lhsT_layout(w)
        hi = wl.astype(F16NP)
        lo = ((wl - hi.astype(f32)) * LSC).astype(F8NP)
        shared[f"{name}_16"] = hi
        shared[f"{name}_8"] = lo
    shared["b2r"] = np.asarray(b2, f32).reshape(1, H).astype(F16NP)
    # tau ACT applies scale=0.5 AFTER the psum seed, so seed the full bg:
    # tanh((pre + bg) * 0.5)
    shared["bgr"] = np.asarray(bg, f32).reshape(1, H).astype(F16NP)
    shared["wh1"] = lhsT_layout(dt * np.asarray(Wh1, f32))
    shared["wh2"] = np.ascontiguousarray(
        np.asarray(Wh2, f32).reshape(KC, 128, O).transpose(1, 0, 2).reshape(128, KC * O)
    )
    shared["bh1t"] = chunk_col(bh1)
    shared["bh2t"] = np.asarray(bh2, f32).reshape(O, 1)

    coeffs = np.asarray(coeffs, f32)  # [B, T, C]
    dW = np.asarray(dW, f32)  # [NT_full, B, H]
    dw_scale = f32(0.5 * sq / dt)

    # exact host-side u_t = x_t @ W1x + b1 for all t, and z0
    x_seq = coeffs.transpose(1, 0, 2)  # [T, B, C]
    u_all = x_seq[:NT] @ W1[H:] + np.asarray(b1, f32)  # [NT, B, H]
    z0 = (x_seq[0] @ np.asarray(W_init, f32) + np.asarray(b_init, f32)) / dt
    shared["eye16"] = np.eye(128, dtype=F16NP)

    def fmaj(a, bs):  # [.., B, H] slice -> [.., 128, KC*BL] feature-major
        v = a[..., bs, :]
        sh = v.shape[:-2]
        v = np.swapaxes(v, -1, -2)  # [.., H, BL]
        v = v.reshape(*sh, KC, 128, BL).swapaxes(-2, -3)  # [.., 128, KC, BL]
        return np.ascontiguousarray(v.reshape(*sh, 128, KC * BL), f32)

    in_maps = []
    for c in range(NCORES):
        bs = slice(c * BL, (c + 1) * BL)
        uf = fmaj(u_all, bs)  # [NT, 128, 128] f32
        uhi = uf.astype(F16NP)
        ulo = (uf - uhi.astype(f32)).astype(F16NP)
        upair = np.concatenate([uhi, ulo], axis=-1)  # [NT, 128, 256] fp16
        in_maps.append(
            {"u": np.ascontiguousarray(upair), "dw": fmaj(dW[:NT] * dw_scale, bs),
             "y0": fmaj(z0, bs), **shared}
        )
    return in_maps


def kernel(**inputs):
    global _BUILT
    if _BUILT is None:
        _BUILT = _build_nc()
    nc = _BUILT
    in_maps = _prep_inputs(**inputs)
    res = run_bass_kernel_spmd(nc, in_maps, core_ids=list(range(NCORES)))
    out = np.empty((B, OUT_TIME, O), np.float32)
    for c, r in enumerate(res.results):
        out[c * BL : (c + 1) * BL] = (
            r["out"].reshape(O, OUT_TIME, BL).transpose(2, 1, 0)
        )
    return out


# revision 5
# speedup vs baseline: 1.1258x; 1.0002x over previous
"""NeuralSDE forecasting kernel for 8x Trainium2 NeuronCores (Bass/Tile). v3.

Data-parallel over batch B=256 across 8 cores (32 batch elems per core).
Feature-major scan: state y.T lives in [128 partitions, 4*32]; column
block k holds features 128k..128k+128 of the 32 local batch columns.

Precision: W = fp16(W) + 2^-11 * e5m2((W - fp16(W)) * 2^11). Per step each
of the three [512,512] products is y_hi@W16 + y_lo@W16 + y8@W8 with fp32
PSUM accumulation (y carried f32, split to fp16 hi/lo; y8 = e5m2 of
y_hi * 2^-11 so the fp8 product scale cancels). CPU-simulated end-to-end
rel err 3.4e-3 (vs 2e-2 budget).

Speed structure (vs the double-bf16 baseline at 1.88ms):
- u_t = x~_t @ [W1x; b1] and z0 are computed EXACTLY on the host and
  shipped as inputs: kills the on-device precompute and its DRAM
  round-trip, the strided per-step u gather, and all u rounding error.
- Both hi products accumulate into a SINGLE [128,32] psum block per
  m-chunk via a broadcast (stride-0) out AP on the N=64 pair matmul --
  verified on HW that both halves accumulate. The fp8 product joins the
  same accumulation group, so no DVE fold of hi/lo psum blocks exists;
  tau and f are computed by ACT reading PSUM directly with per-chunk
  bias APs (saves ~4 DVE hops per step off the critical path).
- The W1y group streams k-outer as three N=32 passes (y_hi, y_lo, y8) so
  its first matmuls depend only on the first 64-column chunk of the new
  state; the tail (f, y16, y2, ylo) is produced in 64-col chunks,
  shrinking the between-step PE stall from ~1.8us to a few hundred ns.

sigmoid(x) = 0.5*(1+tanh(x/2)) keeps the scan on the Tanh ACT table;
0.5 factors are folded into the host-prescaled dW and bg.
"""

import os
import sys

sys.path.insert(0, "/opt/trn_rl_repo")

import numpy as np
import ml_dtypes

import concourse.bass as bass
import concourse.bacc as bacc
import concourse.mybir as mybir
import concourse.tile as tile
from concourse.bass_utils import run_bass_kernel_spmd

B, T, C, H, O = 256, 256, 32, 512, 32
OUT_TIME = 32
NCORES = 8
BL = B // NCORES  # 32 batch elements per core
NT = int(os.environ.get("BASS_NT", T - 1))  # 255 scan steps
SAVE0 = NT - OUT_TIME  # first step whose y_next lands in the output tail
KC = H // 128  # 4 feature chunks
F32 = mybir.dt.float32
F16 = mybir.dt.float16
F8 = mybir.dt.float8e5
F16NP = np.float16
F8NP = ml_dtypes.float8_e5m2
LSC = np.float32(2.0**11)  # lo-split scale

Tanh = mybir.ActivationFunctionType.Tanh
Relu = mybir.ActivationFunctionType.Relu
Copy = mybir.ActivationFunctionType.Copy
Identity = mybir.ActivationFunctionType.Identity

_BUILT = None


def _build_nc():
    nc = bacc.Bacc("TRN2", target_bir_lowering=False, debug=False)

    d_u = nc.dram_tensor("u", [NT, 128, 2 * KC * BL], F16, kind="ExternalInput")
    d_eye = nc.dram_tensor("eye16", [128, 128], F16, kind="ExternalInput")
    d_dw = nc.dram_tensor("dw", [NT, 128, KC * BL], F32, kind="ExternalInput")
    d_y0 = nc.dram_tensor("y0", [128, KC * BL], F32, kind="ExternalInput")
    wnames = ["w1y", "w2", "wg"]
    d_w16 = {
        n: nc.dram_tensor(f"{n}_16", [128, KC * H], F16, kind="ExternalInput")
        for n in wnames
    }
    d_w8 = {
        n: nc.dram_tensor(f"{n}_8", [128, KC * H], F8, kind="ExternalInput")
        for n in wnames
    }
    d_b2r = nc.dram_tensor("b2r", [1, KC * 128], F16, kind="ExternalInput")
    d_bgr = nc.dram_tensor("bgr", [1, KC * 128], F16, kind="ExternalInput")
    d_wh1 = nc.dram_tensor("wh1", [128, KC * H], F32, kind="ExternalInput")
    d_wh2 = nc.dram_tensor("wh2", [128, KC * O], F32, kind="ExternalInput")
    d_bh1 = nc.dram_tensor("bh1t", [128, KC], F32, kind="ExternalInput")
    d_bh2 = nc.dram_tensor("bh2t", [O, 1], F32, kind="ExternalInput")
    d_out = nc.dram_tensor("out", [O, OUT_TIME * BL], F32, kind="ExternalOutput")

    with tile.TileContext(nc) as tc:
        with (
            tc.tile_pool(name="const", bufs=1) as const,
            tc.tile_pool(name="xp", bufs=8) as xp,
            tc.tile_pool(name="dwp", bufs=8) as dwp,
            tc.tile_pool(name="yp", bufs=2) as yp,
            tc.tile_pool(name="tmp", bufs=3) as tmp,
            tc.tile_pool(name="pp", bufs=2, space="PSUM") as pp,
        ):
            # --- resident weights ---
            w16, w8 = {}, {}
            for n in wnames:
                w16[n] = const.tile([128, KC * H], F16, tag=f"w16{n}", name=f"w16_{n}")
                nc.sync.dma_start(out=w16[n][:], in_=d_w16[n][:])
                w8[n] = const.tile([128, KC * H], F8, tag=f"w8{n}", name=f"w8_{n}")
                nc.sync.dma_start(out=w8[n][:], in_=d_w8[n][:])
            b2r = const.tile([1, KC * 128], F16, tag="b2r")
            bgr = const.tile([1, KC * 128], F16, tag="bgr")
            ones16 = const.tile([1, BL], F16, tag="ones16")
            nc.vector.memset(ones16[:], 1.0)
            eye16 = const.tile([128, 128], F16, tag="eye16")
            nc.sync.dma_start(out=eye16[:], in_=d_eye[:])
            wh1 = const.tile([128, KC * H], F32, tag="wh1")
            wh2 = const.tile([128, KC * O], F32, tag="wh2")
            bh1 = const.tile([128, KC], F32, tag="bh1")
            bh2 = const.tile([O, 1], F32, tag="bh2")
            slab = const.tile([128, OUT_TIME * 128], F32, tag="slab")
            rT = const.tile([128, KC * 1024], F32, tag="rT")
            outs = const.tile([O, OUT_TIME * BL], F32, tag="outs")
            y0s = const.tile([128, KC * BL], F32, tag="y0s")
            for dst, src in [
                (b2r, d_b2r), (bgr, d_bgr), (wh1, d_wh1), (wh2, d_wh2),
                (bh1, d_bh1), (bh2, d_bh2), (y0s, d_y0),
            ]:
                nc.sync.dma_start(out=dst[:], in_=src[:])

            def wsl(w, n, k, m):  # lhsT tile (k, m) of weight n
                return w[n][:, k * H + m * 128 : k * H + (m + 1) * 128]

            def bcast(ps, m):  # [128, 2, 32] stride-0 view of psum block m
                return (
                    ps[:, m * BL : (m + 1) * BL]
                    .unsqueeze(1)
                    .broadcast_to((128, 2, BL))
                )

            # --- initial state ---
            y = y0s[:]
            yhl = tmp.tile([128, KC * 2 * BL], F16, tag="yhl", name="yhl_init")
            nc.vector.tensor_copy(yhl[:, 0:128], y)
            nc.vector.tensor_sub(yhl[:, 128:256], y, yhl[:, 0:128])
            y8 = tmp.tile([128, KC * BL], F8, tag="y8", name="y8_init")
            nc.scalar.activation(y8[:], yhl[:, 0:128], Copy, scale=float(1.0 / LSC))

            # N=64 pair group (for wg/w2): hi halves via stride-0 bcast out,
            # then the fp8 correction into the same accumulation group.
            # start=True ONLY on the group's first matmul: it clears the
            # has_written bits for the whole psum zero-region; every later
            # matmul relies on per-element has_written (first touch of an
            # element replaces, later touches accumulate). A start=True per
            # m-block would wipe the accumulate flag of sibling blocks.
            def bias_mm(ps, brow):
                # K=1 rank-1 matmul seeding the psum group with a bias row,
                # so tau/f become single full-width ACTs with no chunk bias.
                for m in range(KC):
                    nc.tensor.matmul(
                        ps[:, m * BL : (m + 1) * BL],
                        brow[0:1, m * 128 : (m + 1) * 128], ones16[:],
                        start=(m == 0), stop=False, skip_group_check=True,
                    )

            # C group: N=64 pair matmuls only -- the wg fp8-lo correction is
            # dropped (gate path tolerates fp16-single weights; simulated
            # end-to-end err 0.0152 with the h-single cut below).
            def group_pair(ps, n, hl, lead=True):
                rv = hl[:].rearrange("p (h q) -> p h q", h=2)
                for m in range(KC):
                    bc = bcast(ps, m)
                    for k in range(KC):
                        nc.tensor.matmul(
                            bc, wsl(w16, n, k, m),
                            rv[:, :, k * BL : (k + 1) * BL],
                            start=(lead and m == 0 and k == 0),
                            stop=(m == KC - 1 and k == KC - 1),
                            skip_group_check=True,
                        )

            # B group: h streamed single-fp16 (no h_lo pass) + fp8 leg.
            # psB is SPLIT into two physical psum tiles (separate banks =>
            # separate accumulation groups with their own stop): readers
            # wait on the group-stop instruction, so closing the m0/m1
            # half a half-group early lets the first f chunk start ~430ns
            # before the whole B group finishes.
            def group_b_half(ps, h16, lo8, brow, ms):
                for j, m in enumerate(ms):
                    nc.tensor.matmul(
                        ps[:, j * BL : (j + 1) * BL],
                        brow[0:1, m * 128 : (m + 1) * 128], ones16[:],
                        start=(j == 0), stop=False, skip_group_check=True,
                    )
                for k in range(KC):
                    for j, m in enumerate(ms):
                        nc.tensor.matmul(
                            ps[:, j * BL : (j + 1) * BL], wsl(w16, "w2", k, m),
                            h16[:, k * BL : (k + 1) * BL],
                            start=False, stop=False, skip_group_check=True,
                        )
                for k in range(KC):
                    for j, m in enumerate(ms):
                        nc.tensor.matmul(
                            ps[:, j * BL : (j + 1) * BL], wsl(w8, "w2", k, m),
                            lo8[:, k * BL : (k + 1) * BL],
                            start=False, stop=(k == KC - 1 and j == len(ms) - 1),
                            skip_group_check=True,
                        )

            # k-outer 3-pass group ordered by when the tail produces each
            # input: y16 pass, then y8 (fp8) pass, then ylo pass -- the
            # DVE/ACT tail emits y16 chunks, y8, then ylo chunks, so
            # consumption matches production and the mid-group stall that
            # used to wait on ylo disappears.
            def group_split(ph, n, hl, lo8):
                # ph = (psA0, psA1): half-tiles close independently so the
                # h16 chunks start as soon as their half-group stops.
                def blk(m):
                    return ph[m // 2][:, (m % 2) * BL : (m % 2 + 1) * BL]
                for k in range(KC):
                    for m in range(KC):
                        nc.tensor.matmul(
                            blk(m), wsl(w16, n, k, m),
                            hl[:, k * BL : (k + 1) * BL],
                            start=False, stop=False, skip_group_check=True,
                        )
                for k in range(KC):
                    for m in range(KC):
                        nc.tensor.matmul(
                            blk(m), wsl(w8, n, k, m),
                            lo8[:, k * BL : (k + 1) * BL],
                            start=False, stop=False, skip_group_check=True,
                        )
                for k in range(KC):
                    for m in range(KC):
                        nc.tensor.matmul(
                            blk(m), wsl(w16, n, k, m),
                            hl[:, 128 + k * BL : 128 + (k + 1) * BL],
                            start=False, stop=(k == KC - 1 and m % 2 == 1),
                            skip_group_check=True,
                        )

            # --- scan ---
            for t in range(NT):
                u_t = xp.tile([128, 2 * KC * BL], F16, tag="u", name=f"u_{t}")
                nc.sync.dma_start(out=u_t[:], in_=d_u[t])
                dw_t = dwp.tile([128, KC * BL], F32, tag="dw", name=f"dw_{t}")
                nc.sync.dma_start(out=dw_t[:], in_=d_dw[t])

                # A group: u seeded into psum via two identity matmuls
                # (u shipped as an fp16 hi+lo pair -> exact to 2^-22), so
                # the pre-tanh add needs no DVE op at all and h16 reads
                # PSUM directly.
                psA0 = pp.tile([128, 2 * BL], F32, tag="psA0", bufs=1,
                               name=f"psA0_{t}")
                psA1 = pp.tile([128, 2 * BL], F32, tag="psA1", bufs=1,
                               name=f"psA1_{t}")
                # u (fp16 hi+lo pair) seeded per half-tile via an N=128
                # broadcast matmul (hi then lo accumulate on the same cols)
                uv = u_t[:].rearrange("p (h q) -> p h q", h=2)
                for hf, ph in ((0, psA0), (1, psA1)):
                    nc.tensor.matmul(
                        ph[:].unsqueeze(1).broadcast_to((128, 2, 2 * BL)),
                        eye16[:], uv[:, :, hf * 64 : (hf + 1) * 64],
                        start=True, stop=False, skip_group_check=True,
                    )
                group_split((psA0, psA1), "w1y", yhl, y8)
                # C group emitted immediately after A (inputs all ready), so
                # the h-chain ACTs below overlap C's matmuls.
                psC = pp.tile([128, KC * BL], F32, tag="psC", name=f"psC_{t}")
                bias_mm(psC, bgr)
                group_pair(psC, "wg", yhl, lead=False)

                # h = tanh(y@W1y + u), fp16 single; h8 fp8 leg from h16.
                # chunked: each half reads its psum half-tile at group close
                h16 = tmp.tile([128, KC * BL], F16, tag="h16", name=f"h16_{t}")
                nc.scalar.activation(h16[:, 0:64], psA0[:], Tanh)
                nc.scalar.activation(h16[:, 64:128], psA1[:], Tanh)
                h8 = tmp.tile([128, KC * BL], F8, tag="h8", name=f"h8_{t}")
                nc.scalar.activation(
                    h8[:], h16[:], Copy, scale=float(1.0 / LSC)
                )

                # tau = tanh((y@Wg + bg)/2)  (sigmoid fold; bias pre-seeded)
                tau = tmp.tile([128, KC * BL], F32, tag="tau", name=f"tau_{t}")
                nc.scalar.activation(tau[:], psC[:], Tanh, scale=0.5)
                # t1 = (tau + 1) * dw ;  dw pre-scaled by 0.5*sqrt(dt)/dt
                t1 = tmp.tile([128, KC * BL], F32, tag="t1", name=f"t1_{t}")
                nc.vector.scalar_tensor_tensor(
                    t1[:], tau[:], 1.0, dw_t[:],
                    mybir.AluOpType.add, mybir.AluOpType.mult,
                )
                yh2 = tmp.tile([128, KC * BL], F32, tag="yh2", name=f"yh2_{t}")
                nc.vector.tensor_add(yh2[:], y, t1[:])

                # B group: f = tanh(h@W2 + b2); h16 single + fp8 leg.
                psB0 = pp.tile([128, 2 * BL], F32, tag="psB0", bufs=2,
                               name=f"psB0_{t}")
                psB1 = pp.tile([128, 2 * BL], F32, tag="psB1", bufs=2,
                               name=f"psB1_{t}")
                group_b_half(psB0, h16, h8, b2r, (0, 1))
                group_b_half(psB1, h16, h8, b2r, (2, 3))
                # f chunks read their half-tile as soon as its group closes
                f = tmp.tile([128, KC * BL], F32, tag="f", name=f"f_{t}")
                nc.scalar.activation(f[:, 0:64], psB0[:], Tanh)
                nc.scalar.activation(f[:, 64:128], psB1[:], Tanh)

                # y_next = (y + t1) + f, produced in 64-col chunks so the
                # next step's first matmuls start after chunk 0.
                if t >= SAVE0:
                    y2 = slab[:, (t - SAVE0) * 128 : (t - SAVE0 + 1) * 128]
                else:
                    y2_t = yp.tile([128, KC * BL], F32, tag="y", name=f"y_{t}")
                    y2 = y2_t[:]
                yhl = tmp.tile([128, KC * 2 * BL], F16, tag="yhl", name=f"yhl_{t}")
                for c in range(2):
                    cs = slice(c * 64, (c + 1) * 64)
                    nc.vector.tensor_add(yhl[:, cs], yh2[:, cs], f[:, cs])
                    nc.vector.tensor_add(y2[:, cs], yh2[:, cs], f[:, cs])
                    nc.vector.tensor_sub(
                        yhl[:, 128 + c * 64 : 128 + (c + 1) * 64],
                        y2[:, cs], yhl[:, cs],
                    )
                y8 = tmp.tile([128, KC * BL], F8, tag="y8", name=f"y8_{t}")
                nc.scalar.activation(
                    y8[:], yhl[:, 0:128], Copy, scale=float(1.0 / LSC)
                )
                y = y2

            # --- head (fp32): out = relu(z_tail@Wh1 + bh1) @ Wh2 + bh2 ---
            slab_r = slab[:].rearrange(
                "p (s k b) -> p s k b", s=OUT_TIME, k=KC, b=BL
            )
            for m in range(KC):
                for hf in range(2):
                    ps1 = pp.tile([128, 512], F32, tag="psB0", bufs=2,
                                  name=f"ps1_{m}_{hf}")
                    for k in range(KC):
                        nc.tensor.matmul(
                            ps1[:],
                            wh1[:, k * H + m * 128 : k * H + (m + 1) * 128],
                            slab_r[:, hf * 16 : (hf + 1) * 16, k, :],
                            start=(k == 0), stop=(k == KC - 1),
                        )
                    nc.scalar.activation(
                        rT[:, m * 1024 + hf * 512 : m * 1024 + (hf + 1) * 512],
                        ps1[:], Relu, bias=bh1[:, m : m + 1],
                    )
            for hf in range(2):
                ps2 = pp.tile([O, 512], F32, tag="psC", name=f"ps2_{hf}")
                for m in range(KC):
                    nc.tensor.matmul(
                        ps2[:],
                        wh2[:, m * O : (m + 1) * O],
                        rT[:, m * 1024 + hf * 512 : m * 1024 + (hf + 1) * 512],
                        start=(m == 0), stop=(m == KC - 1),
                    )
                nc.scalar.activation(
                    outs[:, hf * 512 : (hf + 1) * 512], ps2[:], Identity,
                    bias=bh2[:],
                )
            nc.sync.dma_start(out=d_out[:], in_=outs[:])

    nc.compile()
    return nc


def _prep_inputs(times, coeffs, final_index, dW, W_init, b_init, W1, b1, W2,
                 b2, Wg, bg, Wh1, bh1, Wh2, bh2):
    f32 = np.float32
    times = np.asarray(times, f32)
    dt = f32(max(np.min(times[1:] - times[:-1]), f32(0.001)))
    sq = f32(np.sqrt(dt))

    def lhsT_layout(w):  # [H, H] -> [128, KC*H] with (k,m) tile at k*H+m*128
        return np.ascontiguousarray(
            np.asarray(w, f32).reshape(KC, 128, H).transpose(1, 0, 2).reshape(128, KC * H)
        )

    def chunk_col(b):  # [H] -> [128, KC]
        return np.ascontiguousarray(np.asarray(b, f32).reshape(KC, 128).T)

    W1 = np.asarray(W1, f32)
    shared = {}
    for name, w in [("w1y", dt * W1[:H]), ("w2", np.asarray(W2, f32)),
                    ("wg", dt * np.asarray(Wg, f32))]:
        wl = lhsT_layout(w)
        hi = wl.astype(F16NP)
        lo = ((wl - hi.astype(f32)) * LSC).astype(F8NP)
        shared[f"{name}_16"] = hi
        shared[f"{name}_8"] = lo
    shared["b2r"] = np.asarray(b2, f32).reshape(1, H).astype(F16NP)
    # tau ACT applies scale=0.5 AFTER the psum seed, so seed the full bg:
    # tanh((pre + bg) * 0.5)
    shared["bgr"] = np.asarray(bg, f32).reshape(1, H).astype(F16NP)
    shared["wh1"] = lhsT_layout(dt * np.asarray(Wh1, f32))
    shared["wh2"] = np.ascontiguousarray(
        np.asarray(Wh2, f32).reshape(KC, 128, O).transpose(1, 0, 2).reshape(128, KC * O)
    )
    shared["bh1t"] = chunk_col(bh1)
    shared["bh2t"] = np.asarray(bh2, f32).reshape(O, 1)

    coeffs = np.asarray(coeffs, f32)  # [B, T, C]
    dW = np.asarray(dW, f32)  # [NT_full, B, H]
    dw_scale = f32(0.5 * sq / dt)

    # exact host-side u_t = x_t @ W1x + b1 for all t, and z0
    x_seq = coeffs.transpose(1, 0, 2)  # [T, B, C]
    u_all = x_seq[:NT] @ W1[H:] + np.asarray(b1, f32)  # [NT, B, H]
    z0 = (x_seq[0] @ np.asarray(W_init, f32) + np.asarray(b_init, f32)) / dt
    shared["eye16"] = np.eye(128, dtype=F16NP)

    def fmaj(a, bs):  # [.., B, H] slice -> [.., 128, KC*BL] feature-major
        v = a[..., bs, :]
        sh = v.shape[:-2]
        v = np.swapaxes(v, -1, -2)  # [.., H, BL]
        v = v.reshape(*sh, KC, 128, BL).swapaxes(-2, -3)  # [.., 128, KC, BL]
        return np.ascontiguousarray(v.reshape(*sh, 128, KC * BL), f32)

    in_maps = []
    for c in range(NCORES):
        bs = slice(c * BL, (c + 1) * BL)
        uf = fmaj(u_all, bs)  # [NT, 128, 128] f32
        uhi = uf.astype(F16NP)
        ulo = (uf - uhi.astype(f32)).astype(F16NP)
        upair = np.concatenate([uhi, ulo], axis=-1)  # [NT, 128, 256] fp16
        in_maps.append(
            {"u": np.ascontiguousarray(upair), "dw": fmaj(dW[:NT] * dw_scale, bs),
             "y0": fmaj(z0, bs), **shared}
        )
    return in_maps


def kernel(**inputs):
    global _BUILT
    if _BUILT is None:
        _BUILT = _build_nc()
    nc = _BUILT
    in_maps = _prep_inputs(**inputs)
    res = run_bass_kernel_spmd(nc, in_maps, core_ids=list(range(NCORES)))
    out = np.empty((B, OUT_TIME, O), np.float32)
    for c, r in enumerate(res.results):
        out[c * BL : (c + 1) * BL] = (
            r["out"].reshape(O, OUT_TIME, BL).transpose(2, 1, 0)
        )
    return out


# revision 6
# speedup vs baseline: 1.1614x; 1.0317x over previous
"""NeuralSDE forecasting kernel for 8x Trainium2 NeuronCores (Bass/Tile). v3.

Data-parallel over batch B=256 across 8 cores (32 batch elems per core).
Feature-major scan: state y.T lives in [128 partitions, 4*32]; column
block k holds features 128k..128k+128 of the 32 local batch columns.

Precision: W = fp16(W) + 2^-11 * e5m2((W - fp16(W)) * 2^11). Per step each
of the three [512,512] products is y_hi@W16 + y_lo@W16 + y8@W8 with fp32
PSUM accumulation (y carried f32, split to fp16 hi/lo; y8 = e5m2 of
y_hi * 2^-11 so the fp8 product scale cancels). CPU-simulated end-to-end
rel err 3.4e-3 (vs 2e-2 budget).

Speed structure (vs the double-bf16 baseline at 1.88ms):
- u_t = x~_t @ [W1x; b1] and z0 are computed EXACTLY on the host and
  shipped as inputs: kills the on-device precompute and its DRAM
  round-trip, the strided per-step u gather, and all u rounding error.
- Both hi products accumulate into a SINGLE [128,32] psum block per
  m-chunk via a broadcast (stride-0) out AP on the N=64 pair matmul --
  verified on HW that both halves accumulate. The fp8 product joins the
  same accumulation group, so no DVE fold of hi/lo psum blocks exists;
  tau and f are computed by ACT reading PSUM directly with per-chunk
  bias APs (saves ~4 DVE hops per step off the critical path).
- The W1y group streams k-outer as three N=32 passes (y_hi, y_lo, y8) so
  its first matmuls depend only on the first 64-column chunk of the new
  state; the tail (f, y16, y2, ylo) is produced in 64-col chunks,
  shrinking the between-step PE stall from ~1.8us to a few hundred ns.

sigmoid(x) = 0.5*(1+tanh(x/2)) keeps the scan on the Tanh ACT table;
0.5 factors are folded into the host-prescaled dW and bg.
"""

import os
import sys

sys.path.insert(0, "/opt/trn_rl_repo")

import numpy as np
import ml_dtypes

import concourse.bass as bass
import concourse.bacc as bacc
import concourse.mybir as mybir
import concourse.tile as tile
from concourse.bass_utils import run_bass_kernel_spmd

B, T, C, H, O = 256, 256, 32, 512, 32
OUT_TIME = 32
NCORES = 8
BL = B // NCORES  # 32 batch elements per core
NT = int(os.environ.get("BASS_NT", T - 1))  # 255 scan steps
SAVE0 = NT - OUT_TIME  # first step whose y_next lands in the output tail
KC = H // 128  # 4 feature chunks
F32 = mybir.dt.float32
F16 = mybir.dt.float16
F8 = mybir.dt.float8e5
F16NP = np.float16
F8NP = ml_dtypes.float8_e5m2
LSC = np.float32(2.0**11)  # lo-split scale

Tanh = mybir.ActivationFunctionType.Tanh
Relu = mybir.ActivationFunctionType.Relu
Copy = mybir.ActivationFunctionType.Copy
Identity = mybir.ActivationFunctionType.Identity

_BUILT = None


def _build_nc():
    nc = bacc.Bacc("TRN2", target_bir_lowering=False, debug=False)

    d_u = nc.dram_tensor("u", [NT, 128, 2 * KC * BL], F16, kind="ExternalInput")
    d_eye = nc.dram_tensor("eye16", [128, 128], F16, kind="ExternalInput")
    d_dw = nc.dram_tensor("dw", [NT, 128, KC * BL], F32, kind="ExternalInput")
    d_y0 = nc.dram_tensor("y0", [128, KC * BL], F32, kind="ExternalInput")
    wnames = ["w1y", "w2", "wg"]
    d_w16 = {
        n: nc.dram_tensor(f"{n}_16", [128, KC * H], F16, kind="ExternalInput")
        for n in wnames
    }
    d_w8 = {
        n: nc.dram_tensor(f"{n}_8", [128, KC * H], F8, kind="ExternalInput")
        for n in wnames
    }
    d_b2r = nc.dram_tensor("b2r", [1, KC * 128], F16, kind="ExternalInput")
    d_bgr = nc.dram_tensor("bgr", [1, KC * 128], F16, kind="ExternalInput")
    d_wh1 = nc.dram_tensor("wh1", [128, KC * H], F32, kind="ExternalInput")
    d_wh2 = nc.dram_tensor("wh2", [128, KC * O], F32, kind="ExternalInput")
    d_bh1 = nc.dram_tensor("bh1t", [128, KC], F32, kind="ExternalInput")
    d_bh2 = nc.dram_tensor("bh2t", [O, 1], F32, kind="ExternalInput")
    d_out = nc.dram_tensor("out", [O, OUT_TIME * BL], F32, kind="ExternalOutput")

    with tile.TileContext(nc) as tc:
        with (
            tc.tile_pool(name="const", bufs=1) as const,
            tc.tile_pool(name="xp", bufs=8) as xp,
            tc.tile_pool(name="dwp", bufs=8) as dwp,
            tc.tile_pool(name="yp", bufs=2) as yp,
            tc.tile_pool(name="tmp", bufs=3) as tmp,
            tc.tile_pool(name="pp", bufs=2, space="PSUM") as pp,
        ):
            # --- resident weights ---
            w16, w8 = {}, {}
            for n in wnames:
                w16[n] = const.tile([128, KC * H], F16, tag=f"w16{n}", name=f"w16_{n}")
                nc.sync.dma_start(out=w16[n][:], in_=d_w16[n][:])
                w8[n] = const.tile([128, KC * H], F8, tag=f"w8{n}", name=f"w8_{n}")
                nc.sync.dma_start(out=w8[n][:], in_=d_w8[n][:])
            b2r = const.tile([1, KC * 128], F16, tag="b2r")
            bgr = const.tile([1, KC * 128], F16, tag="bgr")
            ones16 = const.tile([1, BL], F16, tag="ones16")
            nc.vector.memset(ones16[:], 1.0)
            eye16 = const.tile([128, 128], F16, tag="eye16")
            nc.sync.dma_start(out=eye16[:], in_=d_eye[:])
            wh1 = const.tile([128, KC * H], F32, tag="wh1")
            wh2 = const.tile([128, KC * O], F32, tag="wh2")
            bh1 = const.tile([128, KC], F32, tag="bh1")
            bh2 = const.tile([O, 1], F32, tag="bh2")
            slab = const.tile([128, OUT_TIME * 128], F32, tag="slab")
            rT = const.tile([128, KC * 1024], F32, tag="rT")
            outs = const.tile([O, OUT_TIME * BL], F32, tag="outs")
            y0s = const.tile([128, KC * BL], F32, tag="y0s")
            for dst, src in [
                (b2r, d_b2r), (bgr, d_bgr), (wh1, d_wh1), (wh2, d_wh2),
                (bh1, d_bh1), (bh2, d_bh2), (y0s, d_y0),
            ]:
                nc.sync.dma_start(out=dst[:], in_=src[:])

            def wsl(w, n, k, m):  # lhsT tile (k, m) of weight n
                return w[n][:, k * H + m * 128 : k * H + (m + 1) * 128]

            def bcast(ps, m):  # [128, 2, 32] stride-0 view of psum block m
                return (
                    ps[:, m * BL : (m + 1) * BL]
                    .unsqueeze(1)
                    .broadcast_to((128, 2, BL))
                )

            # --- initial state ---
            y = y0s[:]
            yhl = tmp.tile([128, KC * 2 * BL], F16, tag="yhl", name="yhl_init")
            nc.vector.tensor_copy(yhl[:, 0:128], y)
            nc.vector.tensor_sub(yhl[:, 128:256], y, yhl[:, 0:128])
            y8 = tmp.tile([128, KC * BL], F8, tag="y8", name="y8_init")
            nc.scalar.activation(y8[:], yhl[:, 0:128], Copy, scale=float(1.0 / LSC))

            # N=64 pair group (for wg/w2): hi halves via stride-0 bcast out,
            # then the fp8 correction into the same accumulation group.
            # start=True ONLY on the group's first matmul: it clears the
            # has_written bits for the whole psum zero-region; every later
            # matmul relies on per-element has_written (first touch of an
            # element replaces, later touches accumulate). A start=True per
            # m-block would wipe the accumulate flag of sibling blocks.
            def bias_mm(ps, brow):
                # K=1 rank-1 matmul seeding the psum group with a bias row,
                # so tau/f become single full-width ACTs with no chunk bias.
                for m in range(KC):
                    nc.tensor.matmul(
                        ps[:, m * BL : (m + 1) * BL],
                        brow[0:1, m * 128 : (m + 1) * 128], ones16[:],
                        start=(m == 0), stop=False, skip_group_check=True,
                    )

            # C group: N=64 pair matmuls only -- the wg fp8-lo correction is
            # dropped (gate path tolerates fp16-single weights; simulated
            # end-to-end err 0.0152 with the h-single cut below).
            def group_pair(ps, n, hl, lead=True):
                rv = hl[:].rearrange("p (h q) -> p h q", h=2)
                for m in range(KC):
                    bc = bcast(ps, m)
                    for k in range(KC):
                        nc.tensor.matmul(
                            bc, wsl(w16, n, k, m),
                            rv[:, :, k * BL : (k + 1) * BL],
                            start=(lead and m == 0 and k == 0),
                            stop=(m == KC - 1 and k == KC - 1),
                            skip_group_check=True,
                        )

            # B group: h streamed single-fp16 (no h_lo pass) + fp8 leg.
            # psB is SPLIT into two physical psum tiles (separate banks =>
            # separate accumulation groups with their own stop): readers
            # wait on the group-stop instruction, so closing the m0/m1
            # half a half-group early lets the first f chunk start ~430ns
            # before the whole B group finishes.
            def group_b_half(ps, h16, lo8, brow, ms):
                for j, m in enumerate(ms):
                    nc.tensor.matmul(
                        ps[:, j * BL : (j + 1) * BL],
                        brow[0:1, m * 128 : (m + 1) * 128], ones16[:],
                        start=(j == 0), stop=False, skip_group_check=True,
                    )
                for k in range(KC):
                    for j, m in enumerate(ms):
                        nc.tensor.matmul(
                            ps[:, j * BL : (j + 1) * BL], wsl(w16, "w2", k, m),
                            h16[:, k * BL : (k + 1) * BL],
                            start=False, stop=False, skip_group_check=True,
                        )
                for k in range(KC):
                    for j, m in enumerate(ms):
                        nc.tensor.matmul(
                            ps[:, j * BL : (j + 1) * BL], wsl(w8, "w2", k, m),
                            lo8[:, k * BL : (k + 1) * BL],
                            start=False, stop=(k == KC - 1 and j == len(ms) - 1),
                            skip_group_check=True,
                        )

            # k-outer 3-pass group ordered by when the tail produces each
            # input: y16 pass, then y8 (fp8) pass, then ylo pass -- the
            # DVE/ACT tail emits y16 chunks, y8, then ylo chunks, so
            # consumption matches production and the mid-group stall that
            # used to wait on ylo disappears.
            def group_split(ph, n, hl, lo8):
                # ph = (psA0, psA1): half-tiles close independently so the
                # h16 chunks start as soon as their half-group stops.
                def blk(m):
                    return ph[m // 2][:, (m % 2) * BL : (m % 2 + 1) * BL]
                for k in range(KC):
                    for m in range(KC):
                        nc.tensor.matmul(
                            blk(m), wsl(w16, n, k, m),
                            hl[:, k * BL : (k + 1) * BL],
                            start=False, stop=False, skip_group_check=True,
                        )
                for k in range(KC):
                    for m in range(KC):
                        nc.tensor.matmul(
                            blk(m), wsl(w8, n, k, m),
                            lo8[:, k * BL : (k + 1) * BL],
                            start=False, stop=False, skip_group_check=True,
                        )
                for k in range(KC):
                    for m in range(KC):
                        nc.tensor.matmul(
                            blk(m), wsl(w16, n, k, m),
                            hl[:, 128 + k * BL : 128 + (k + 1) * BL],
                            start=False, stop=(k == KC - 1 and m % 2 == 1),
                            skip_group_check=True,
                        )

            # --- scan ---
            for t in range(NT):
                u_t = xp.tile([128, 2 * KC * BL], F16, tag="u", name=f"u_{t}")
                nc.sync.dma_start(out=u_t[:], in_=d_u[t])
                dw_t = dwp.tile([128, KC * BL], F32, tag="dw", name=f"dw_{t}")
                nc.sync.dma_start(out=dw_t[:], in_=d_dw[t])

                # A group: u seeded into psum via two identity matmuls
                # (u shipped as an fp16 hi+lo pair -> exact to 2^-22), so
                # the pre-tanh add needs no DVE op at all and h16 reads
                # PSUM directly.
                psA0 = pp.tile([128, 2 * BL], F32, tag="psA0", bufs=1,
                               name=f"psA0_{t}")
                psA1 = pp.tile([128, 2 * BL], F32, tag="psA1", bufs=1,
                               name=f"psA1_{t}")
                # u (fp16 hi+lo pair) seeded per half-tile via an N=128
                # broadcast matmul (hi then lo accumulate on the same cols)
                uv = u_t[:].rearrange("p (h q) -> p h q", h=2)
                for hf, ph in ((0, psA0), (1, psA1)):
                    nc.tensor.matmul(
                        ph[:].unsqueeze(1).broadcast_to((128, 2, 2 * BL)),
                        eye16[:], uv[:, :, hf * 64 : (hf + 1) * 64],
                        start=True, stop=False, skip_group_check=True,
                    )
                group_split((psA0, psA1), "w1y", yhl, y8)
                # C group emitted immediately after A (inputs all ready), so
                # the h-chain ACTs below overlap C's matmuls.
                psC0 = pp.tile([128, 2 * BL], F32, tag="psC0", bufs=1,
                               name=f"psC0_{t}")
                psC1 = pp.tile([128, 2 * BL], F32, tag="psC1", bufs=1,
                               name=f"psC1_{t}")
                for hf, pc in ((0, psC0), (1, psC1)):
                    for j in range(2):
                        m = 2 * hf + j
                        nc.tensor.matmul(
                            pc[:, j * BL : (j + 1) * BL],
                            bgr[0:1, m * 128 : (m + 1) * 128], ones16[:],
                            start=(j == 0), stop=False, skip_group_check=True,
                        )
                    rv = yhl[:].rearrange("p (h q) -> p h q", h=2)
                    for j in range(2):
                        m = 2 * hf + j
                        bc = (
                            pc[:, j * BL : (j + 1) * BL]
                            .unsqueeze(1)
                            .broadcast_to((128, 2, BL))
                        )
                        for k in range(KC):
                            nc.tensor.matmul(
                                bc, wsl(w16, "wg", k, m),
                                rv[:, :, k * BL : (k + 1) * BL],
                                start=False,
                                stop=(j == 1 and k == KC - 1),
                                skip_group_check=True,
                            )

                # h = tanh(y@W1y + u), fp16 single; h8 fp8 leg from h16.
                # chunked: each half reads its psum half-tile at group close
                h16 = tmp.tile([128, KC * BL], F16, tag="h16", name=f"h16_{t}")
                nc.scalar.activation(h16[:, 0:64], psA0[:], Tanh)
                nc.scalar.activation(h16[:, 64:128], psA1[:], Tanh)
                # h8 on DVE (exact, verified) to decongest the ACT queue
                h8 = tmp.tile([128, KC * BL], F8, tag="h8", name=f"h8_{t}")
                nc.vector.tensor_scalar_mul(h8[:], h16[:], float(1.0 / LSC))

                # tau = tanh((y@Wg + bg)/2), chunked per psC half-tile;
                # t1/yh2 chunked to match so yh2 chunk 0 lands early.
                tau = tmp.tile([128, KC * BL], F32, tag="tau", name=f"tau_{t}")
                t1 = tmp.tile([128, KC * BL], F32, tag="t1", name=f"t1_{t}")
                yh2 = tmp.tile([128, KC * BL], F32, tag="yh2", name=f"yh2_{t}")
                for hf, pc in ((0, psC0), (1, psC1)):
                    cs = slice(hf * 64, (hf + 1) * 64)
                    nc.scalar.activation(tau[:, cs], pc[:], Tanh, scale=0.5)
                    nc.vector.scalar_tensor_tensor(
                        t1[:, cs], tau[:, cs], 1.0, dw_t[:, cs],
                        mybir.AluOpType.add, mybir.AluOpType.mult,
                    )
                    nc.vector.tensor_add(yh2[:, cs], y[:, cs], t1[:, cs])

                # B group: f = tanh(h@W2 + b2); h16 single + fp8 leg.
                psB0 = pp.tile([128, 2 * BL], F32, tag="psB0", bufs=2,
                               name=f"psB0_{t}")
                psB1 = pp.tile([128, 2 * BL], F32, tag="psB1", bufs=2,
                               name=f"psB1_{t}")
                group_b_half(psB0, h16, h8, b2r, (0, 1))
                group_b_half(psB1, h16, h8, b2r, (2, 3))
                # f chunks read their half-tile as soon as its group closes
                f = tmp.tile([128, KC * BL], F32, tag="f", name=f"f_{t}")
                nc.scalar.activation(f[:, 0:64], psB0[:], Tanh)
                nc.scalar.activation(f[:, 64:128], psB1[:], Tanh)

                # y_next = (y + t1) + f, produced in 64-col chunks so the
                # next step's first matmuls start after chunk 0.
                if t >= SAVE0:
                    y2 = slab[:, (t - SAVE0) * 128 : (t - SAVE0 + 1) * 128]
                else:
                    y2_t = yp.tile([128, KC * BL], F32, tag="y", name=f"y_{t}")
                    y2 = y2_t[:]
                yhl = tmp.tile([128, KC * 2 * BL], F16, tag="yhl", name=f"yhl_{t}")
                for c in range(2):
                    cs = slice(c * 64, (c + 1) * 64)
                    nc.vector.tensor_add(yhl[:, cs], yh2[:, cs], f[:, cs])
                    nc.vector.tensor_add(y2[:, cs], yh2[:, cs], f[:, cs])
                    nc.vector.tensor_sub(
                        yhl[:, 128 + c * 64 : 128 + (c + 1) * 64],
                        y2[:, cs], yhl[:, cs],
                    )
                y8 = tmp.tile([128, KC * BL], F8, tag="y8", name=f"y8_{t}")
                nc.scalar.activation(
                    y8[:], yhl[:, 0:128], Copy, scale=float(1.0 / LSC)
                )
                y = y2

            # --- head (fp32): out = relu(z_tail@Wh1 + bh1) @ Wh2 + bh2 ---
            slab_r = slab[:].rearrange(
                "p (s k b) -> p s k b", s=OUT_TIME, k=KC, b=BL
            )
            for m in range(KC):
                for hf in range(2):
                    ps1 = pp.tile([128, 512], F32, tag="psB0", bufs=2,
                                  name=f"ps1_{m}_{hf}")
                    for k in range(KC):
                        nc.tensor.matmul(
                            ps1[:],
                            wh1[:, k * H + m * 128 : k * H + (m + 1) * 128],
                            slab_r[:, hf * 16 : (hf + 1) * 16, k, :],
                            start=(k == 0), stop=(k == KC - 1),
                        )
                    nc.scalar.activation(
                        rT[:, m * 1024 + hf * 512 : m * 1024 + (hf + 1) * 512],
                        ps1[:], Relu, bias=bh1[:, m : m + 1],
                    )
            for hf in range(2):
                ps2 = pp.tile([O, 512], F32, tag="psB1", bufs=2, name=f"ps2_{hf}")
                for m in range(KC):
                    nc.tensor.matmul(
                        ps2[:],
                        wh2[:, m * O : (m + 1) * O],
                        rT[:, m * 1024 + hf * 512 : m * 1024 + (hf + 1) * 512],
                        start=(m == 0), stop=(m == KC - 1),
                    )
                nc.scalar.activation(
                    outs[:, hf * 512 : (hf + 1) * 512], ps2[:], Identity,
                    bias=bh2[:],
                )
            nc.sync.dma_start(out=d_out[:], in_=outs[:])

    nc.compile()
    return nc


def _prep_inputs(times, coeffs, final_index, dW, W_init, b_init, W1, b1, W2,
                 b2, Wg, bg, Wh1, bh1, Wh2, bh2):
    f32 = np.float32
    times = np.asarray(times, f32)
    dt = f32(max(np.min(times[1:] - times[:-1]), f32(0.001)))
    sq = f32(np.sqrt(dt))

    def lhsT_layout(w):  # [H, H] -> [128, KC*H] with (k,m) tile at k*H+m*128
        return np.ascontiguousarray(
            np.asarray(w, f32).reshape(KC, 128, H).transpose(1, 0, 2).reshape(128, KC * H)
        )

    def chunk_col(b):  # [H] -> [128, KC]
        return np.ascontiguousarray(np.asarray(b, f32).reshape(KC, 128).T)

    W1 = np.asarray(W1, f32)
    shared = {}
    for name, w in [("w1y", dt * W1[:H]), ("w2", np.asarray(W2, f32)),
                    ("wg", dt * np.asarray(Wg, f32))]:
        wl = lhsT_layout(w)
        hi = wl.astype(F16NP)
        lo = ((wl - hi.astype(f32)) * LSC).astype(F8NP)
        shared[f"{name}_16"] = hi
        shared[f"{name}_8"] = lo
    shared["b2r"] = np.asarray(b2, f32).reshape(1, H).astype(F16NP)
    # tau ACT applies scale=0.5 AFTER the psum seed, so seed the full bg:
    # tanh((pre + bg) * 0.5)
    shared["bgr"] = np.asarray(bg, f32).reshape(1, H).astype(F16NP)
    shared["wh1"] = lhsT_layout(dt * np.asarray(Wh1, f32))
    shared["wh2"] = np.ascontiguousarray(
        np.asarray(Wh2, f32).reshape(KC, 128, O).transpose(1, 0, 2).reshape(128, KC * O)
    )
    shared["bh1t"] = chunk_col(bh1)
    shared["bh2t"] = np.asarray(bh2, f32).reshape(O, 1)

    coeffs = np.asarray(coeffs, f32)  # [B, T, C]
    dW = np.asarray(dW, f32)  # [NT_full, B, H]
    dw_scale = f32(0.5 * sq / dt)

    # exact host-side u_t = x_t @ W1x + b1 for all t, and z0
    x_seq = coeffs.transpose(1, 0, 2)  # [T, B, C]
    u_all = x_seq[:NT] @ W1[H:] + np.asarray(b1, f32)  # [NT, B, H]
    z0 = (x_seq[0] @ np.asarray(W_init, f32) + np.asarray(b_init, f32)) / dt
    shared["eye16"] = np.eye(128, dtype=F16NP)

    def fmaj(a, bs):  # [.., B, H] slice -> [.., 128, KC*BL] feature-major
        v = a[..., bs, :]
        sh = v.shape[:-2]
        v = np.swapaxes(v, -1, -2)  # [.., H, BL]
        v = v.reshape(*sh, KC, 128, BL).swapaxes(-2, -3)  # [.., 128, KC, BL]
        return np.ascontiguousarray(v.reshape(*sh, 128, KC * BL), f32)

    in_maps = []
    for c in range(NCORES):
        bs = slice(c * BL, (c + 1) * BL)
        uf = fmaj(u_all, bs)  # [NT, 128, 128] f32
        uhi = uf.astype(F16NP)
        ulo = (uf - uhi.astype(f32)).astype(F16NP)
        upair = np.concatenate([uhi, ulo], axis=-1)  # [NT, 128, 256] fp16
        in_maps.append(
            {"u": np.ascontiguousarray(upair), "dw": fmaj(dW[:NT] * dw_scale, bs),
             "y0": fmaj(z0, bs), **shared}
        )
    return in_maps


def kernel(**inputs):
    global _BUILT
    if _BUILT is None:
        _BUILT = _build_nc()
    nc = _BUILT
    in_maps = _prep_inputs(**inputs)
    res = run_bass_kernel_spmd(nc, in_maps, core_ids=list(range(NCORES)))
    out = np.empty((B, OUT_TIME, O), np.float32)
    for c, r in enumerate(res.results):
        out[c * BL : (c + 1) * BL] = (
            r["out"].reshape(O, OUT_TIME, BL).transpose(2, 1, 0)
        )
    return out
